# revision 10
# baseline (speedup 1.0000x reference)
"""Trainium2 Bass kernel for nn_DistangledLearn (scatter_memory).

Strategy (8 NeuronCores, SPMD, no collectives):
  * Sharding by cluster: the host relabels/assigns clusters to cores so each
    core owns exactly 256 clusters (8 of them its share of the 64 target
    clusters, placed in window slot 0) and its instance rows arrive sorted
    by window.  Cluster->window packing is load-balanced (greedy LPT + swap
    refinement) so nearly all 8-cluster windows fit in 2 tiles of 128 rows
    (~68 tiles/core vs 96 for the naive fixed schedule).
  * Per 128-row tile the PE computes sums[d, cols] += data.T @ onehot where
    the one-hot ships as fp8_e4m3 (exact for 0/1, half the bytes of bf16)
    and data ships bf16.  Mixed bf16xfp8 matmul is supported by the PE.
  * Per 8-window block (1024 bucket cols): PSUM sums are evacuated to SBUF
    by the VectorE, the PE computes dots = inputs @ sums (f32r) and bucket
    norms^2 via ones-matmuls over ScalarE-squared sums.  Only dots
    [64, 4096]->bf16, norms^2 [1,4096]->f32 and the window-0 (target
    clusters) sums [128,256]->f32 are shipped back - the full [128, 8192]
    sums stay on-chip (the old kernel shipped them: 2 MB/core).
  * Host does the tiny O(B*C) assembly: counts via bincount, positive
    prototypes from the window-0 sums, cluster-prototype softmax, negative
    exp-sums from dots*scale(norms), and the final scalar loss.
"""
import numpy as np

N, D, C, K, R, B = 65536, 256, 2048, 2, 8, 64
DATA_SCALE = 16.0
TEMP, TAU, EPS = 0.05, 0.5, 1e-12
NC = 8
CP = C // NC          # clusters per core = 256
WIN = 8               # clusters per window
NWIN = CP // WIN      # window slots per core = 32
NBLK = 4              # psum blocks of 8 window slots
P = 128


# ----------------------------------------------------------------------------
# host-side cluster assignment + packing
# ----------------------------------------------------------------------------

def _lpt_assign(items, sizes, nbins, cap):
    """Greedy LPT: assign items (desc by size) to the lightest bin with
    remaining capacity. Returns (bins, bsum)."""
    order = np.argsort(-sizes, kind="stable")
    bins = [[] for _ in range(nbins)]
    bsum = [0.0] * nbins
    cnt = [0] * nbins
    for i in order:
        b = min((bb for bb in range(nbins) if cnt[bb] < cap),
                key=lambda bb: bsum[bb])
        bins[b].append(int(items[i]))
        bsum[b] += float(sizes[i])
        cnt[b] += 1
    return bins, np.asarray(bsum)


def _refine(bins, bsum, szmap, limit=256, passes=40):
    """Swap items between over-limit and under-limit bins to push as many
    bins as possible under `limit` (deterministic hill-climb)."""
    nb = len(bins)
    for _ in range(passes):
        improved = False
        over = sorted((b for b in range(nb) if bsum[b] > limit),
                      key=lambda b: -bsum[b])
        if not over:
            break
        under = [b for b in range(nb) if bsum[b] < limit]
        for b1 in over:
            best = None
            for b2 in under:
                for i1, c1 in enumerate(bins[b1]):
                    for i2, c2 in enumerate(bins[b2]):
                        d = szmap[c1] - szmap[c2]
                        if d <= 0:
                            continue
                        if bsum[b1] - d <= limit and bsum[b2] + d <= limit:
                            best = (b2, i1, i2, d)
                            break
                    if best:
                        break
                if best:
                    break
            if best:
                b2, i1, i2, d = best
                c1, c2 = bins[b1][i1], bins[b2][i2]
                bins[b1][i1], bins[b2][i2] = c2, c1
                bsum[b1] -= d
                bsum[b2] += d
                improved = True
        if not improved:
            break
    return bins, bsum


def host_prep(labels, irre, targets):
    """Cluster->core/window assignment, tile schedule, per-core device inputs
    metadata.

    Returns dict with:
      sched   [32] int     tiles per window slot (shared by all cores)
      gidx    [NC, P, NT]  int64 row index into the instance bank (N = pad)
      ohcol   [NC, P, NT, K] int16 one-hot col within window (-1 = pad)
      core_of [C], slot_of [C], cl_of [C]   cluster -> (core, slot, pos)
    """
    labels = np.asarray(labels).astype(np.int64)
    irre = np.asarray(irre).astype(np.int64)
    targets = np.asarray(targets).astype(np.int64)
    sz = np.bincount(labels, minlength=C).astype(np.int64)

    # 1) eight target clusters per core (window slot 0)
    tbins, tsum = _lpt_assign(targets, sz[targets], NC, 8)

    # 2) remaining clusters -> cores, 248 each, balancing row totals
    rest = np.setdiff1d(np.arange(C), targets)
    order = np.argsort(-sz[rest], kind="stable")
    core_cl = [list(tbins[c]) for c in range(NC)]
    core_sum = [float(tsum[c]) for c in range(NC)]
    cnt = [0] * NC
    for i in order:
        cl = int(rest[i])
        c = min((cc for cc in range(NC) if cnt[cc] < CP - 8),
                key=lambda cc: core_sum[cc])
        core_cl[c].append(cl)
        core_sum[c] += float(sz[cl])
        cnt[c] += 1

    # 3) per core: pack the 248 non-target clusters into 31 windows of 8
    szmap = {int(c): int(s) for c, s in enumerate(sz)}
    core_windows = []          # [NC][32] -> list of 8 cluster ids
    for c in range(NC):
        nont = core_cl[c][8:]
        bins, bsum = _lpt_assign(np.asarray(nont), sz[nont], NWIN - 1, WIN)
        bins, bsum = _refine(bins, bsum, szmap)
        order_w = np.argsort(-bsum, kind="stable")
        wins = [list(tbins[c])] + [bins[i] for i in order_w]
        core_windows.append(wins)

    # 4) shared tile schedule: max tiles needed at each slot across cores
    rows_cw = np.zeros((NC, NWIN), np.int64)
    for c in range(NC):
        for s in range(NWIN):
            rows_cw[c, s] = sum(szmap[cl] for cl in core_windows[c][s])
    sched = np.maximum(np.ceil(rows_cw / P).astype(np.int64).max(axis=0), 1)
    NT = int(sched.sum())
    tbase = np.zeros(NWIN + 1, np.int64)
    np.cumsum(sched, out=tbase[1:])

    # 5) row layout + one-hot codes
    core_of = np.zeros(C, np.int64)
    slot_of = np.zeros(C, np.int64)
    cl_of = np.zeros(C, np.int64)
    for c in range(NC):
        for s in range(NWIN):
            for q, cl in enumerate(core_windows[c][s]):
                core_of[cl] = c
                slot_of[cl] = s
                cl_of[cl] = q

    # rows of each cluster (grouped): order rows by (core, slot, cluster)
    sort_key = (core_of[labels] * NWIN + slot_of[labels]) * C + labels
    row_order = np.argsort(sort_key, kind="stable").astype(np.int64)
    slab = labels[row_order]
    score = core_of[slab]
    sslot = slot_of[slab]

    # position within (core, slot)
    cw_id = score * NWIN + sslot
    starts = np.zeros(NC * NWIN + 1, np.int64)
    np.cumsum(np.bincount(cw_id, minlength=NC * NWIN), out=starts[1:])
    j = np.arange(N, dtype=np.int64) - starts[cw_id]
    tile_in_w, prow = np.divmod(j, P)
    t = tbase[sslot] + tile_in_w

    gidx = np.full((NC, P, NT), N, dtype=np.int64)
    gidx[score, prow, t] = row_order
    ohcol = np.full((NC, P, NT, K), -1, dtype=np.int64)
    clw = cl_of[slab]
    for k in range(K):
        ohcol[score, prow, t, k] = clw * 16 + k * 8 + irre[row_order, k]

    return dict(sched=sched, NT=NT, tbase=tbase, gidx=gidx, ohcol=ohcol,
                core_of=core_of, slot_of=slot_of, cl_of=cl_of,
                core_targets=[list(tbins[c]) for c in range(NC)])


# ----------------------------------------------------------------------------
# device program
# ----------------------------------------------------------------------------

def build_program(sched):
    from contextlib import ExitStack
    import concourse.bacc as bacc
    import concourse.tile as tile
    from concourse import mybir

    dt = mybir.dt
    sched = [int(x) for x in sched]
    NT = sum(sched)
    tbase = [0]
    for x in sched:
        tbase.append(tbase[-1] + x)
    TW = D + P                # interleaved tile width: 256 data + 128 onehot

    nc = bacc.Bacc("TRN2", target_bir_lowering=False, debug=False,
                   num_devices=NC)

    dat_t = nc.dram_tensor("dat", [P, NT * TW], dt.float8e4,
                           kind="ExternalInput")
    sums_t = nc.dram_tensor("sums", [P, NWIN * 256], dt.float8e4,
                            kind="ExternalOutput")
    win0_t = nc.dram_tensor("win0", [P, 256], dt.bfloat16,
                            kind="ExternalOutput")

    dcuts = [0, 4, 12, 24, 40, 54, 62, NT]
    NHB = NWIN // 4

    with tile.TileContext(nc) as tc, ExitStack() as ctx:
        const = ctx.enter_context(tc.tile_pool(name="const", bufs=1))
        sums_q = const.tile([P, NWIN * 256], dt.float8e4)
        win0_bf = const.tile([P, 256], dt.bfloat16)
        # PE warm-up: keep the HAM activity window busy while inputs stream
        warm = const.tile([P, 256], dt.float8e4)
        nc.gpsimd.memset(warm[:], 0)

        dchunks = []
        for lo, hi in zip(dcuts, dcuts[1:]):
            t = const.tile([P, (hi - lo) * TW], dt.float8e4,
                           name=f"dat{lo}")
            nc.sync.dma_start(out=t[:], in_=dat_t[:, lo * TW:hi * TW])
            dchunks.append((lo, hi, t))

        def dslice(j, ch):
            for lo, hi, t in dchunks:
                if lo <= j < hi:
                    base = (j - lo) * TW
                    return t[:, base + ch * P:base + ch * P + P]
            raise AssertionError
        def oslice(j):
            for lo, hi, t in dchunks:
                if lo <= j < hi:
                    base = (j - lo) * TW
                    return t[:, base + D:base + D + P]
            raise AssertionError

        with tc.tile_pool(name="pseg", bufs=2, space="PSUM") as ppool, \
             tc.tile_pool(name="pwarm", bufs=1, space="PSUM") as wpool:
            wps = wpool.tile([P, 128], dt.float32, tag="wps")
            for _ in range(26):
                nc.tensor.matmul(out=wps[:], lhsT=warm[:, 0:128],
                                 rhs=warm[:, 128:256], start=True, stop=True)
            for h in range(NHB):
                slots = sched[h * 4:(h + 1) * 4]
                ps0 = ppool.tile([P, 512], dt.float32, tag="ps0", name="ps0")
                ps1 = ppool.tile([P, 512], dt.float32, tag="ps1", name="ps1")
                ps = [ps0, ps1]
                for s4, Ts in enumerate(slots):
                    j0 = tbase[h * 4 + s4]
                    for i in range(Ts):
                        rhs = oslice(j0 + i)
                        for ch in range(2):
                            nc.tensor.matmul(
                                out=ps[ch][:, s4 * P:(s4 + 1) * P],
                                lhsT=dslice(j0 + i, ch),
                                rhs=rhs,
                                start=(i == 0),
                                stop=(i == Ts - 1),
                            )
                hb = h * 1024
                nc.vector.tensor_copy(out=sums_q[:, hb:hb + 512], in_=ps0[:])
                nc.scalar.copy(out=sums_q[:, hb + 512:hb + 1024], in_=ps1[:])
                if h == 0:
                    nc.vector.tensor_copy(out=win0_bf[:, 0:128],
                                          in_=ps0[:, 0:128])
                    nc.scalar.copy(out=win0_bf[:, 128:256],
                                   in_=ps1[:, 0:128])
                    nc.scalar.dma_start(out=win0_t[:], in_=win0_bf[:])
                nc.scalar.dma_start(out=sums_t[:, hb:hb + 1024],
                                    in_=sums_q[:, hb:hb + 1024])

    nc.compile()
    return nc


# ----------------------------------------------------------------------------
# glue: shard inputs
# ----------------------------------------------------------------------------

def make_in_maps(inputs_np, ins_np, prep):
    import ml_dtypes
    fp8 = ml_dtypes.float8_e4m3
    NT = prep["NT"]
    gidx = prep["gidx"]
    ohcol = prep["ohcol"]
    TW = D + P

    ins_cast = (ins_np * DATA_SCALE).astype(fp8)
    ins_pad = np.concatenate([ins_cast, np.zeros((1, D), fp8)])

    maps = []
    for c in range(NC):
        idx = gidx[c]                                  # [P, NT]
        dat = np.zeros((P, NT, TW), np.float32)
        dat[:, :, :D] = ins_pad[idx].astype(np.float32)
        for k in range(K):
            col = ohcol[c, :, :, k]
            pp, tt = np.nonzero(col >= 0)
            dat[pp, tt, D + col[pp, tt]] = 1.0
        maps.append({
            "dat": np.ascontiguousarray(dat.reshape(P, NT * TW)).astype(fp8),
        })
    return maps


def run_device(nc, in_maps, trace=False):
    from concourse.bass_utils import run_bass_kernel_spmd
    return run_bass_kernel_spmd(nc, in_maps, list(range(NC)), trace=trace)


# ----------------------------------------------------------------------------
# host-side final assembly
# ----------------------------------------------------------------------------

def host_assemble(inputs, clu, labels, irre, targets, irre_targets, prep,
                  sums_cores, win0_cores):
    labels = np.asarray(labels).astype(np.int64)
    irre = np.asarray(irre).astype(np.int64)
    t = np.asarray(targets).astype(np.int64)
    rt = np.asarray(irre_targets).astype(np.int64)
    inputs = np.asarray(inputs, np.float32)
    clu = np.asarray(clu, np.float32)
    core_of, slot_of, cl_of = prep["core_of"], prep["slot_of"], prep["cl_of"]

    counts_all = np.bincount(labels, minlength=C).astype(np.float32)
    cnt_cr = np.zeros((K, C, R), np.float32)
    for k in range(K):
        cnt_cr[k] = np.bincount(labels * R + irre[:, k],
                                minlength=C * R).reshape(C, R)

    # device col of bucket (cluster, k, r): g = slot*128 + cl*16 + k*8 + r
    gbase = slot_of * 128 + cl_of * 16                     # [C]
    kk_g = np.arange(K)[:, None, None]
    rr_g = np.arange(R)[None, None, :]
    gidx_full = gbase[None, :, None] + kk_g * 8 + rr_g     # [K, C, R]

    # sums ship [P, 8192] fp8: S[ch*128+p, g] where
    #   col = (g//512)*1024 + ch*512 + g%512
    S_cores = np.zeros((NC, 2 * P, NWIN * 128), np.float32)
    for c in range(NC):
        sq = np.asarray(sums_cores[c], np.float32)         # [128, 8192]
        v = sq.reshape(P, NWIN // 4, 2, 512)               # p, hb, ch, cin
        S_cores[c] = (v.transpose(2, 0, 1, 3)
                      .reshape(2 * P, NWIN * 128)) / DATA_SCALE

    norms2 = np.einsum('cdg,cdg->cg', S_cores, S_cores)    # [NC, 4096]
    snorm2 = norms2[core_of[None, :, None], gidx_full]     # [K, C, R]
    snorm = np.sqrt(np.maximum(snorm2, 0.0))

    dots_core = np.einsum('bd,cdg->cbg', inputs, S_cores)  # [NC, B, 4096]
    bb_g = np.arange(B)[:, None, None, None]
    dots_raw = dots_core[core_of[None, None, :, None],
                         bb_g,
                         gidx_full[None]]                  # [B, K, C, R]

    # window-0 sums: per core [128, 256] bf16 -> sums for its 8 targets
    sums_t = np.zeros((B, K, R, D), np.float32)
    tpos = {int(tc): i for i, tc in enumerate(t)}
    for c in range(NC):
        w0 = np.asarray(win0_cores[c], np.float32) / DATA_SCALE   # [P, 256]
        for tc in prep["core_targets"][c]:
            i = tpos[int(tc)]
            colb = cl_of[tc] * 16
            for k in range(K):
                for r in range(R):
                    col = colb + k * 8 + r
                    sums_t[i, k, r] = np.concatenate(
                        [w0[:, col], w0[:, 128 + col]])

    sums_all_t = sums_t[:, 0].sum(axis=1)                  # [B, D]
    kk = np.arange(K)[None, :]
    bb = np.arange(B)[:, None]
    sub_sum = sums_t[bb, kk, rt]                           # [B, K, D]
    sub_cnt = cnt_cr[kk, t[:, None], rt]                   # [B, K]
    pos_sum = sums_all_t[:, None, :] - sub_sum
    pos_cnt = counts_all[t][:, None] - sub_cnt
    has_pos = pos_cnt > 0
    m_pos = np.where(has_pos[..., None],
                     pos_sum / np.maximum(pos_cnt, 1.0)[..., None],
                     clu[t][:, None, :])

    delta_pos = m_pos.sum(axis=1)
    protos = clu.copy()
    protos[t] = (1.0 - TAU) * clu[t] + (TAU / K) * delta_pos
    protos /= np.maximum(np.linalg.norm(protos, axis=1, keepdims=True), EPS)
    outputs = (inputs @ protos.T) / TEMP
    l_pos = np.exp(outputs[np.arange(B), t])
    l_sum = np.exp(outputs).sum(axis=1)

    mcnt = np.maximum(cnt_cr, 1.0)
    mnorm = snorm / mcnt
    scale = 1.0 / (mcnt * np.maximum(mnorm, EPS)) / TEMP   # [K, C, R]
    dots_n = dots_raw * scale[None]

    kk3 = np.arange(K)[None, :, None]
    cc3 = np.arange(C)[None, None, :]
    dots_sel = dots_n[bb[..., None], kk3, cc3, rt[:, :, None]]   # [B, K, C]
    cnt_sel = cnt_cr[kk3, cc3, rt[:, :, None]]
    valid = (cnt_sel > 0) & (cc3 != t[:, None, None])
    delta_neg = np.where(valid, np.exp(dots_sel), 0.0).sum(axis=2)
    any_valid = valid.any(axis=2)
    clu_n = clu / np.maximum(np.linalg.norm(clu, axis=1, keepdims=True), EPS)
    fb = np.exp(np.einsum('bd,bkd->bk', inputs, clu_n[rt]) / TEMP)
    delta = np.where(any_valid, delta_neg, fb)
    l_sum = l_sum + (TAU / K) * delta.sum(axis=1)

    return np.float32(-np.mean(np.log(l_pos / l_sum)))


# ----------------------------------------------------------------------------
# entry point
# ----------------------------------------------------------------------------

def kernel(**inputs):
    inputs_np = np.asarray(inputs["inputs"], np.float32)
    ins_np = np.ascontiguousarray(np.asarray(inputs["ins_memory"], np.float32))
    clu_np = np.asarray(inputs["clu_memory"], np.float32)
    labels = np.asarray(inputs["labels"])
    irre = np.asarray(inputs["irre_labels"])
    targets = np.asarray(inputs["targets"])
    irre_targets = np.asarray(inputs["irre_targets"])

    prep = host_prep(labels, irre, targets)
    nc = build_program(prep["sched"])
    in_maps = make_in_maps(inputs_np, ins_np, prep)
    res = run_device(nc, in_maps)
    sums_cores = [r["sums"] for r in res.results]
    win0_cores = [r["win0"] for r in res.results]
    return host_assemble(inputs_np, clu_np, labels, irre, targets,
                         irre_targets, prep, sums_cores, win0_cores)


# revision 11
# speedup vs baseline: 1.0431x; 1.0431x over previous
"""Trainium2 Bass kernel for nn_DistangledLearn (scatter_memory).

Strategy (8 NeuronCores, SPMD, no collectives):
  * Sharding by cluster: the host relabels/assigns clusters to cores so each
    core owns exactly 256 clusters (8 of them its share of the 64 target
    clusters, placed in window slot 0) and its instance rows arrive sorted
    by window.  Cluster->window packing is load-balanced (greedy LPT + swap
    refinement) so nearly all 8-cluster windows fit in 2 tiles of 128 rows
    (~68 tiles/core vs 96 for the naive fixed schedule).
  * Per 128-row tile the PE computes sums[d, cols] += data.T @ onehot where
    the one-hot ships as fp8_e4m3 (exact for 0/1, half the bytes of bf16)
    and data ships bf16.  Mixed bf16xfp8 matmul is supported by the PE.
  * Per 8-window block (1024 bucket cols): PSUM sums are evacuated to SBUF
    by the VectorE, the PE computes dots = inputs @ sums (f32r) and bucket
    norms^2 via ones-matmuls over ScalarE-squared sums.  Only dots
    [64, 4096]->bf16, norms^2 [1,4096]->f32 and the window-0 (target
    clusters) sums [128,256]->f32 are shipped back - the full [128, 8192]
    sums stay on-chip (the old kernel shipped them: 2 MB/core).
  * Host does the tiny O(B*C) assembly: counts via bincount, positive
    prototypes from the window-0 sums, cluster-prototype softmax, negative
    exp-sums from dots*scale(norms), and the final scalar loss.
"""
import numpy as np

N, D, C, K, R, B = 65536, 256, 2048, 2, 8, 64
DATA_SCALE = 16.0
TEMP, TAU, EPS = 0.05, 0.5, 1e-12
NC = 8
CP = C // NC          # clusters per core = 256
WIN = 8               # clusters per window
NWIN = CP // WIN      # window slots per core = 32
NBLK = 4              # psum blocks of 8 window slots
P = 128


# ----------------------------------------------------------------------------
# host-side cluster assignment + packing
# ----------------------------------------------------------------------------

def _lpt_assign(items, sizes, nbins, cap):
    """Greedy LPT: assign items (desc by size) to the lightest bin with
    remaining capacity. Returns (bins, bsum)."""
    order = np.argsort(-sizes, kind="stable")
    bins = [[] for _ in range(nbins)]
    bsum = [0.0] * nbins
    cnt = [0] * nbins
    for i in order:
        b = min((bb for bb in range(nbins) if cnt[bb] < cap),
                key=lambda bb: bsum[bb])
        bins[b].append(int(items[i]))
        bsum[b] += float(sizes[i])
        cnt[b] += 1
    return bins, np.asarray(bsum)


def _refine(bins, bsum, szmap, limit=256, passes=40):
    """Swap items between over-limit and under-limit bins to push as many
    bins as possible under `limit` (deterministic hill-climb)."""
    nb = len(bins)
    for _ in range(passes):
        improved = False
        over = sorted((b for b in range(nb) if bsum[b] > limit),
                      key=lambda b: -bsum[b])
        if not over:
            break
        under = [b for b in range(nb) if bsum[b] < limit]
        for b1 in over:
            best = None
            for b2 in under:
                for i1, c1 in enumerate(bins[b1]):
                    for i2, c2 in enumerate(bins[b2]):
                        d = szmap[c1] - szmap[c2]
                        if d <= 0:
                            continue
                        if bsum[b1] - d <= limit and bsum[b2] + d <= limit:
                            best = (b2, i1, i2, d)
                            break
                    if best:
                        break
                if best:
                    break
            if best:
                b2, i1, i2, d = best
                c1, c2 = bins[b1][i1], bins[b2][i2]
                bins[b1][i1], bins[b2][i2] = c2, c1
                bsum[b1] -= d
                bsum[b2] += d
                improved = True
        if not improved:
            break
    return bins, bsum


def host_prep(labels, irre, targets):
    """Cluster->core/window assignment, tile schedule, per-core device inputs
    metadata.

    Returns dict with:
      sched   [32] int     tiles per window slot (shared by all cores)
      gidx    [NC, P, NT]  int64 row index into the instance bank (N = pad)
      ohcol   [NC, P, NT, K] int16 one-hot col within window (-1 = pad)
      core_of [C], slot_of [C], cl_of [C]   cluster -> (core, slot, pos)
    """
    labels = np.asarray(labels).astype(np.int64)
    irre = np.asarray(irre).astype(np.int64)
    targets = np.asarray(targets).astype(np.int64)
    sz = np.bincount(labels, minlength=C).astype(np.int64)

    # 1) eight target clusters per core (window slot 0)
    tbins, tsum = _lpt_assign(targets, sz[targets], NC, 8)

    # 2) remaining clusters -> cores, 248 each, balancing row totals
    rest = np.setdiff1d(np.arange(C), targets)
    order = np.argsort(-sz[rest], kind="stable")
    core_cl = [list(tbins[c]) for c in range(NC)]
    core_sum = [float(tsum[c]) for c in range(NC)]
    cnt = [0] * NC
    for i in order:
        cl = int(rest[i])
        c = min((cc for cc in range(NC) if cnt[cc] < CP - 8),
                key=lambda cc: core_sum[cc])
        core_cl[c].append(cl)
        core_sum[c] += float(sz[cl])
        cnt[c] += 1

    # 3) per core: pack the 248 non-target clusters into 31 windows of 8
    szmap = {int(c): int(s) for c, s in enumerate(sz)}
    core_windows = []          # [NC][32] -> list of 8 cluster ids
    for c in range(NC):
        nont = core_cl[c][8:]
        bins, bsum = _lpt_assign(np.asarray(nont), sz[nont], NWIN - 1, WIN)
        bins, bsum = _refine(bins, bsum, szmap)
        order_w = np.argsort(-bsum, kind="stable")
        wins = [list(tbins[c])] + [bins[i] for i in order_w]
        core_windows.append(wins)

    # 4) shared tile schedule: max tiles needed at each slot across cores
    rows_cw = np.zeros((NC, NWIN), np.int64)
    for c in range(NC):
        for s in range(NWIN):
            rows_cw[c, s] = sum(szmap[cl] for cl in core_windows[c][s])
    sched = np.maximum(np.ceil(rows_cw / P).astype(np.int64).max(axis=0), 1)
    NT = int(sched.sum())
    tbase = np.zeros(NWIN + 1, np.int64)
    np.cumsum(sched, out=tbase[1:])

    # 5) row layout + one-hot codes
    core_of = np.zeros(C, np.int64)
    slot_of = np.zeros(C, np.int64)
    cl_of = np.zeros(C, np.int64)
    for c in range(NC):
        for s in range(NWIN):
            for q, cl in enumerate(core_windows[c][s]):
                core_of[cl] = c
                slot_of[cl] = s
                cl_of[cl] = q

    # rows of each cluster (grouped): order rows by (core, slot, cluster)
    sort_key = (core_of[labels] * NWIN + slot_of[labels]) * C + labels
    row_order = np.argsort(sort_key, kind="stable").astype(np.int64)
    slab = labels[row_order]
    score = core_of[slab]
    sslot = slot_of[slab]

    # position within (core, slot)
    cw_id = score * NWIN + sslot
    starts = np.zeros(NC * NWIN + 1, np.int64)
    np.cumsum(np.bincount(cw_id, minlength=NC * NWIN), out=starts[1:])
    j = np.arange(N, dtype=np.int64) - starts[cw_id]
    tile_in_w, prow = np.divmod(j, P)
    t = tbase[sslot] + tile_in_w

    gidx = np.full((NC, P, NT), N, dtype=np.int64)
    gidx[score, prow, t] = row_order
    ohcol = np.full((NC, P, NT, K), -1, dtype=np.int64)
    clw = cl_of[slab]
    for k in range(K):
        ohcol[score, prow, t, k] = clw * 16 + k * 8 + irre[row_order, k]

    return dict(sched=sched, NT=NT, tbase=tbase, gidx=gidx, ohcol=ohcol,
                core_of=core_of, slot_of=slot_of, cl_of=cl_of,
                core_targets=[list(tbins[c]) for c in range(NC)])


# ----------------------------------------------------------------------------
# device program
# ----------------------------------------------------------------------------

def build_program(sched):
    from contextlib import ExitStack
    import concourse.bacc as bacc
    import concourse.tile as tile
    from concourse import mybir

    dt = mybir.dt
    sched = [int(x) for x in sched]
    NT = sum(sched)
    tbase = [0]
    for x in sched:
        tbase.append(tbase[-1] + x)
    TW = D + P                # interleaved tile width: 256 data + 128 onehot

    nc = bacc.Bacc("TRN2", target_bir_lowering=False, debug=False,
                   num_devices=NC)

    dat_t = nc.dram_tensor("dat", [P, NT * TW], dt.float8e4,
                           kind="ExternalInput")
    sums_t = nc.dram_tensor("sums", [P, NWIN * 256], dt.float8e4,
                            kind="ExternalOutput")
    win0_t = nc.dram_tensor("win0", [P, 256], dt.bfloat16,
                            kind="ExternalOutput")

    dcuts = [0, 8, 20, 40, 56, NT]
    NHB = NWIN // 4

    with tile.TileContext(nc) as tc, ExitStack() as ctx:
        const = ctx.enter_context(tc.tile_pool(name="const", bufs=1))
        sums_q = const.tile([P, NWIN * 256], dt.float8e4)
        win0_bf = const.tile([P, 256], dt.bfloat16)
        # PE warm-up: keep the HAM activity window busy while inputs stream
        warm = const.tile([P, 256], dt.float8e4)
        nc.gpsimd.memset(warm[:], 0)

        dchunks = []
        for lo, hi in zip(dcuts, dcuts[1:]):
            t = const.tile([P, (hi - lo) * TW], dt.float8e4,
                           name=f"dat{lo}")
            nc.sync.dma_start(out=t[:], in_=dat_t[:, lo * TW:hi * TW])
            dchunks.append((lo, hi, t))

        def dslice(j, ch):
            for lo, hi, t in dchunks:
                if lo <= j < hi:
                    base = (j - lo) * TW
                    return t[:, base + ch * P:base + ch * P + P]
            raise AssertionError
        def oslice(j):
            for lo, hi, t in dchunks:
                if lo <= j < hi:
                    base = (j - lo) * TW
                    return t[:, base + D:base + D + P]
            raise AssertionError

        with tc.tile_pool(name="pseg", bufs=2, space="PSUM") as ppool, \
             tc.tile_pool(name="pwarm", bufs=1, space="PSUM") as wpool:
            wps = wpool.tile([P, 128], dt.float32, tag="wps")
            for _ in range(26):
                nc.tensor.matmul(out=wps[:], lhsT=warm[:, 0:128],
                                 rhs=warm[:, 128:256], start=True, stop=True)
            for h in range(NHB):
                slots = sched[h * 4:(h + 1) * 4]
                ps0 = ppool.tile([P, 512], dt.float32, tag="ps0", name="ps0")
                ps1 = ppool.tile([P, 512], dt.float32, tag="ps1", name="ps1")
                ps = [ps0, ps1]
                for s4, Ts in enumerate(slots):
                    j0 = tbase[h * 4 + s4]
                    for i in range(Ts):
                        rhs = oslice(j0 + i)
                        for ch in range(2):
                            nc.tensor.matmul(
                                out=ps[ch][:, s4 * P:(s4 + 1) * P],
                                lhsT=dslice(j0 + i, ch),
                                rhs=rhs,
                                start=(i == 0),
                                stop=(i == Ts - 1),
                            )
                hb = h * 1024
                nc.vector.tensor_copy(out=sums_q[:, hb:hb + 512], in_=ps0[:])
                nc.scalar.copy(out=sums_q[:, hb + 512:hb + 1024], in_=ps1[:])
                if h == 0:
                    nc.vector.tensor_copy(out=win0_bf[:, 0:128],
                                          in_=ps0[:, 0:128])
                    nc.scalar.copy(out=win0_bf[:, 128:256],
                                   in_=ps1[:, 0:128])
                    nc.gpsimd.dma_start(out=win0_t[:], in_=win0_bf[:])
                if h % 2 == 1:
                    qb = (h - 1) * 1024
                    nc.gpsimd.dma_start(out=sums_t[:, qb:qb + 2048],
                                        in_=sums_q[:, qb:qb + 2048])

    nc.compile()
    return nc


# ----------------------------------------------------------------------------
# glue: shard inputs
# ----------------------------------------------------------------------------

def make_in_maps(inputs_np, ins_np, prep):
    import ml_dtypes
    fp8 = ml_dtypes.float8_e4m3
    NT = prep["NT"]
    gidx = prep["gidx"]
    ohcol = prep["ohcol"]
    TW = D + P

    ins_cast = (ins_np * DATA_SCALE).astype(fp8)
    ins_pad = np.concatenate([ins_cast, np.zeros((1, D), fp8)])

    maps = []
    for c in range(NC):
        idx = gidx[c]                                  # [P, NT]
        dat = np.zeros((P, NT, TW), np.float32)
        dat[:, :, :D] = ins_pad[idx].astype(np.float32)
        for k in range(K):
            col = ohcol[c, :, :, k]
            pp, tt = np.nonzero(col >= 0)
            dat[pp, tt, D + col[pp, tt]] = 1.0
        maps.append({
            "dat": np.ascontiguousarray(dat.reshape(P, NT * TW)).astype(fp8),
        })
    return maps


def run_device(nc, in_maps, trace=False):
    from concourse.bass_utils import run_bass_kernel_spmd
    return run_bass_kernel_spmd(nc, in_maps, list(range(NC)), trace=trace)


# ----------------------------------------------------------------------------
# host-side final assembly
# ----------------------------------------------------------------------------

def host_assemble(inputs, clu, labels, irre, targets, irre_targets, prep,
                  sums_cores, win0_cores):
    labels = np.asarray(labels).astype(np.int64)
    irre = np.asarray(irre).astype(np.int64)
    t = np.asarray(targets).astype(np.int64)
    rt = np.asarray(irre_targets).astype(np.int64)
    inputs = np.asarray(inputs, np.float32)
    clu = np.asarray(clu, np.float32)
    core_of, slot_of, cl_of = prep["core_of"], prep["slot_of"], prep["cl_of"]

    counts_all = np.bincount(labels, minlength=C).astype(np.float32)
    cnt_cr = np.zeros((K, C, R), np.float32)
    for k in range(K):
        cnt_cr[k] = np.bincount(labels * R + irre[:, k],
                                minlength=C * R).reshape(C, R)

    # device col of bucket (cluster, k, r): g = slot*128 + cl*16 + k*8 + r
    gbase = slot_of * 128 + cl_of * 16                     # [C]
    kk_g = np.arange(K)[:, None, None]
    rr_g = np.arange(R)[None, None, :]
    gidx_full = gbase[None, :, None] + kk_g * 8 + rr_g     # [K, C, R]

    # sums ship [P, 8192] fp8: S[ch*128+p, g] where
    #   col = (g//512)*1024 + ch*512 + g%512
    S_cores = np.zeros((NC, 2 * P, NWIN * 128), np.float32)
    for c in range(NC):
        sq = np.asarray(sums_cores[c], np.float32)         # [128, 8192]
        v = sq.reshape(P, NWIN // 4, 2, 512)               # p, hb, ch, cin
        S_cores[c] = (v.transpose(2, 0, 1, 3)
                      .reshape(2 * P, NWIN * 128)) / DATA_SCALE

    norms2 = np.einsum('cdg,cdg->cg', S_cores, S_cores)    # [NC, 4096]
    snorm2 = norms2[core_of[None, :, None], gidx_full]     # [K, C, R]
    snorm = np.sqrt(np.maximum(snorm2, 0.0))

    dots_core = np.einsum('bd,cdg->cbg', inputs, S_cores)  # [NC, B, 4096]
    bb_g = np.arange(B)[:, None, None, None]
    dots_raw = dots_core[core_of[None, None, :, None],
                         bb_g,
                         gidx_full[None]]                  # [B, K, C, R]

    # window-0 sums: per core [128, 256] bf16 -> sums for its 8 targets
    sums_t = np.zeros((B, K, R, D), np.float32)
    tpos = {int(tc): i for i, tc in enumerate(t)}
    for c in range(NC):
        w0 = np.asarray(win0_cores[c], np.float32) / DATA_SCALE   # [P, 256]
        for tc in prep["core_targets"][c]:
            i = tpos[int(tc)]
            colb = cl_of[tc] * 16
            for k in range(K):
                for r in range(R):
                    col = colb + k * 8 + r
                    sums_t[i, k, r] = np.concatenate(
                        [w0[:, col], w0[:, 128 + col]])

    sums_all_t = sums_t[:, 0].sum(axis=1)                  # [B, D]
    kk = np.arange(K)[None, :]
    bb = np.arange(B)[:, None]
    sub_sum = sums_t[bb, kk, rt]                           # [B, K, D]
    sub_cnt = cnt_cr[kk, t[:, None], rt]                   # [B, K]
    pos_sum = sums_all_t[:, None, :] - sub_sum
    pos_cnt = counts_all[t][:, None] - sub_cnt
    has_pos = pos_cnt > 0
    m_pos = np.where(has_pos[..., None],
                     pos_sum / np.maximum(pos_cnt, 1.0)[..., None],
                     clu[t][:, None, :])

    delta_pos = m_pos.sum(axis=1)
    protos = clu.copy()
    protos[t] = (1.0 - TAU) * clu[t] + (TAU / K) * delta_pos
    protos /= np.maximum(np.linalg.norm(protos, axis=1, keepdims=True), EPS)
    outputs = (inputs @ protos.T) / TEMP
    l_pos = np.exp(outputs[np.arange(B), t])
    l_sum = np.exp(outputs).sum(axis=1)

    mcnt = np.maximum(cnt_cr, 1.0)
    mnorm = snorm / mcnt
    scale = 1.0 / (mcnt * np.maximum(mnorm, EPS)) / TEMP   # [K, C, R]
    dots_n = dots_raw * scale[None]

    kk3 = np.arange(K)[None, :, None]
    cc3 = np.arange(C)[None, None, :]
    dots_sel = dots_n[bb[..., None], kk3, cc3, rt[:, :, None]]   # [B, K, C]
    cnt_sel = cnt_cr[kk3, cc3, rt[:, :, None]]
    valid = (cnt_sel > 0) & (cc3 != t[:, None, None])
    delta_neg = np.where(valid, np.exp(dots_sel), 0.0).sum(axis=2)
    any_valid = valid.any(axis=2)
    clu_n = clu / np.maximum(np.linalg.norm(clu, axis=1, keepdims=True), EPS)
    fb = np.exp(np.einsum('bd,bkd->bk', inputs, clu_n[rt]) / TEMP)
    delta = np.where(any_valid, delta_neg, fb)
    l_sum = l_sum + (TAU / K) * delta.sum(axis=1)

    return np.float32(-np.mean(np.log(l_pos / l_sum)))


# ----------------------------------------------------------------------------
# entry point
# ----------------------------------------------------------------------------

def kernel(**inputs):
    inputs_np = np.asarray(inputs["inputs"], np.float32)
    ins_np = np.ascontiguousarray(np.asarray(inputs["ins_memory"], np.float32))
    clu_np = np.asarray(inputs["clu_memory"], np.float32)
    labels = np.asarray(inputs["labels"])
    irre = np.asarray(inputs["irre_labels"])
    targets = np.asarray(inputs["targets"])
    irre_targets = np.asarray(inputs["irre_targets"])

    prep = host_prep(labels, irre, targets)
    nc = build_program(prep["sched"])
    in_maps = make_in_maps(inputs_np, ins_np, prep)
    res = run_device(nc, in_maps)
    sums_cores = [r["sums"] for r in res.results]
    win0_cores = [r["win0"] for r in res.results]
    return host_assemble(inputs_np, clu_np, labels, irre, targets,
                         irre_targets, prep, sums_cores, win0_cores)


# revision 12
# speedup vs baseline: 1.0768x; 1.0323x over previous
"""Trainium2 Bass kernel for nn_DistangledLearn (scatter_memory).

Strategy (8 NeuronCores, SPMD, no collectives):
  * Sharding by cluster: the host relabels/assigns clusters to cores so each
    core owns exactly 256 clusters (8 of them its share of the 64 target
    clusters, placed in window slot 0) and its instance rows arrive sorted
    by window.  Cluster->window packing is load-balanced (greedy LPT + swap
    refinement) so nearly all 8-cluster windows fit in 2 tiles of 128 rows
    (~68 tiles/core vs 96 for the naive fixed schedule).
  * Per 128-row tile the PE computes sums[d, cols] += data.T @ onehot where
    the one-hot ships as fp8_e4m3 (exact for 0/1, half the bytes of bf16)
    and data ships bf16.  Mixed bf16xfp8 matmul is supported by the PE.
  * Per 8-window block (1024 bucket cols): PSUM sums are evacuated to SBUF
    by the VectorE, the PE computes dots = inputs @ sums (f32r) and bucket
    norms^2 via ones-matmuls over ScalarE-squared sums.  Only dots
    [64, 4096]->bf16, norms^2 [1,4096]->f32 and the window-0 (target
    clusters) sums [128,256]->f32 are shipped back - the full [128, 8192]
    sums stay on-chip (the old kernel shipped them: 2 MB/core).
  * Host does the tiny O(B*C) assembly: counts via bincount, positive
    prototypes from the window-0 sums, cluster-prototype softmax, negative
    exp-sums from dots*scale(norms), and the final scalar loss.
"""
import numpy as np

N, D, C, K, R, B = 65536, 256, 2048, 2, 8, 64
DATA_SCALE = 16.0
TEMP, TAU, EPS = 0.05, 0.5, 1e-12
NC = 8
CP = C // NC          # clusters per core = 256
WIN = 8               # clusters per window
NWIN = CP // WIN      # window slots per core = 32
NBLK = 4              # psum blocks of 8 window slots
P = 128


# ----------------------------------------------------------------------------
# host-side cluster assignment + packing
# ----------------------------------------------------------------------------

def _lpt_assign(items, sizes, nbins, cap):
    """Greedy LPT: assign items (desc by size) to the lightest bin with
    remaining capacity. Returns (bins, bsum)."""
    order = np.argsort(-sizes, kind="stable")
    bins = [[] for _ in range(nbins)]
    bsum = [0.0] * nbins
    cnt = [0] * nbins
    for i in order:
        b = min((bb for bb in range(nbins) if cnt[bb] < cap),
                key=lambda bb: bsum[bb])
        bins[b].append(int(items[i]))
        bsum[b] += float(sizes[i])
        cnt[b] += 1
    return bins, np.asarray(bsum)


def _refine(bins, bsum, szmap, limit=256, passes=40):
    """Swap items between over-limit and under-limit bins to push as many
    bins as possible under `limit` (deterministic hill-climb)."""
    nb = len(bins)
    for _ in range(passes):
        improved = False
        over = sorted((b for b in range(nb) if bsum[b] > limit),
                      key=lambda b: -bsum[b])
        if not over:
            break
        under = [b for b in range(nb) if bsum[b] < limit]
        for b1 in over:
            best = None
            for b2 in under:
                for i1, c1 in enumerate(bins[b1]):
                    for i2, c2 in enumerate(bins[b2]):
                        d = szmap[c1] - szmap[c2]
                        if d <= 0:
                            continue
                        if bsum[b1] - d <= limit and bsum[b2] + d <= limit:
                            best = (b2, i1, i2, d)
                            break
                    if best:
                        break
                if best:
                    break
            if best:
                b2, i1, i2, d = best
                c1, c2 = bins[b1][i1], bins[b2][i2]
                bins[b1][i1], bins[b2][i2] = c2, c1
                bsum[b1] -= d
                bsum[b2] += d
                improved = True
        if not improved:
            break
    return bins, bsum


def host_prep(labels, irre, targets):
    """Cluster->core/window assignment, tile schedule, per-core device inputs
    metadata.

    Returns dict with:
      sched   [32] int     tiles per window slot (shared by all cores)
      gidx    [NC, P, NT]  int64 row index into the instance bank (N = pad)
      ohcol   [NC, P, NT, K] int16 one-hot col within window (-1 = pad)
      core_of [C], slot_of [C], cl_of [C]   cluster -> (core, slot, pos)
    """
    labels = np.asarray(labels).astype(np.int64)
    irre = np.asarray(irre).astype(np.int64)
    targets = np.asarray(targets).astype(np.int64)
    sz = np.bincount(labels, minlength=C).astype(np.int64)

    # 1) eight target clusters per core (window slot 0)
    tbins, tsum = _lpt_assign(targets, sz[targets], NC, 8)

    # 2) remaining clusters -> cores, 248 each, balancing row totals
    rest = np.setdiff1d(np.arange(C), targets)
    order = np.argsort(-sz[rest], kind="stable")
    core_cl = [list(tbins[c]) for c in range(NC)]
    core_sum = [float(tsum[c]) for c in range(NC)]
    cnt = [0] * NC
    for i in order:
        cl = int(rest[i])
        c = min((cc for cc in range(NC) if cnt[cc] < CP - 8),
                key=lambda cc: core_sum[cc])
        core_cl[c].append(cl)
        core_sum[c] += float(sz[cl])
        cnt[c] += 1

    # 3) per core: pack the 248 non-target clusters into 31 windows of 8
    szmap = {int(c): int(s) for c, s in enumerate(sz)}
    core_windows = []          # [NC][32] -> list of 8 cluster ids
    for c in range(NC):
        nont = core_cl[c][8:]
        bins, bsum = _lpt_assign(np.asarray(nont), sz[nont], NWIN - 1, WIN)
        bins, bsum = _refine(bins, bsum, szmap)
        order_w = np.argsort(-bsum, kind="stable")
        wins = [list(tbins[c])] + [bins[i] for i in order_w]
        core_windows.append(wins)

    # 4) shared tile schedule: max tiles needed at each slot across cores
    rows_cw = np.zeros((NC, NWIN), np.int64)
    for c in range(NC):
        for s in range(NWIN):
            rows_cw[c, s] = sum(szmap[cl] for cl in core_windows[c][s])
    sched = np.maximum(np.ceil(rows_cw / P).astype(np.int64).max(axis=0), 1)
    NT = int(sched.sum())
    tbase = np.zeros(NWIN + 1, np.int64)
    np.cumsum(sched, out=tbase[1:])

    # 5) row layout + one-hot codes
    core_of = np.zeros(C, np.int64)
    slot_of = np.zeros(C, np.int64)
    cl_of = np.zeros(C, np.int64)
    for c in range(NC):
        for s in range(NWIN):
            for q, cl in enumerate(core_windows[c][s]):
                core_of[cl] = c
                slot_of[cl] = s
                cl_of[cl] = q

    # rows of each cluster (grouped): order rows by (core, slot, cluster)
    sort_key = (core_of[labels] * NWIN + slot_of[labels]) * C + labels
    row_order = np.argsort(sort_key, kind="stable").astype(np.int64)
    slab = labels[row_order]
    score = core_of[slab]
    sslot = slot_of[slab]

    # position within (core, slot)
    cw_id = score * NWIN + sslot
    starts = np.zeros(NC * NWIN + 1, np.int64)
    np.cumsum(np.bincount(cw_id, minlength=NC * NWIN), out=starts[1:])
    j = np.arange(N, dtype=np.int64) - starts[cw_id]
    tile_in_w, prow = np.divmod(j, P)
    t = tbase[sslot] + tile_in_w

    gidx = np.full((NC, P, NT), N, dtype=np.int64)
    gidx[score, prow, t] = row_order
    ohcol = np.full((NC, P, NT, K), -1, dtype=np.int64)
    clw = cl_of[slab]
    for k in range(K):
        ohcol[score, prow, t, k] = clw * 16 + k * 8 + irre[row_order, k]

    return dict(sched=sched, NT=NT, tbase=tbase, gidx=gidx, ohcol=ohcol,
                core_of=core_of, slot_of=slot_of, cl_of=cl_of,
                core_targets=[list(tbins[c]) for c in range(NC)])


# ----------------------------------------------------------------------------
# device program
# ----------------------------------------------------------------------------

def build_program(sched):
    from contextlib import ExitStack
    import concourse.bacc as bacc
    import concourse.tile as tile
    from concourse import mybir

    dt = mybir.dt
    sched = [int(x) for x in sched]
    NT = sum(sched)
    tbase = [0]
    for x in sched:
        tbase.append(tbase[-1] + x)
    TW = D + P                # interleaved tile width: 256 data + 128 onehot

    nc = bacc.Bacc("TRN2", target_bir_lowering=False, debug=False,
                   num_devices=NC)

    dat_t = nc.dram_tensor("dat", [P, NT * TW], dt.float8e4,
                           kind="ExternalInput")
    sums_t = nc.dram_tensor("sums", [P, NWIN * 256], dt.float8e4,
                            kind="ExternalOutput")
    win0_t = nc.dram_tensor("win0", [P, 256], dt.bfloat16,
                            kind="ExternalOutput")

    dcuts = [0, 8, 20, 40, 54, 62, NT]
    NHB = NWIN // 4

    with tile.TileContext(nc) as tc, ExitStack() as ctx:
        const = ctx.enter_context(tc.tile_pool(name="const", bufs=1))
        sums_q = const.tile([P, NWIN * 256], dt.float8e4)
        win0_bf = const.tile([P, 256], dt.bfloat16)
        # PE warm-up: keep the HAM activity window busy while inputs stream
        warm = const.tile([P, 256], dt.float8e4)
        nc.gpsimd.memset(warm[:], 0)

        dchunks = []
        for lo, hi in zip(dcuts, dcuts[1:]):
            t = const.tile([P, (hi - lo) * TW], dt.float8e4,
                           name=f"dat{lo}")
            nc.sync.dma_start(out=t[:], in_=dat_t[:, lo * TW:hi * TW])
            dchunks.append((lo, hi, t))

        def dslice(j, ch):
            for lo, hi, t in dchunks:
                if lo <= j < hi:
                    base = (j - lo) * TW
                    return t[:, base + ch * P:base + ch * P + P]
            raise AssertionError
        def oslice(j):
            for lo, hi, t in dchunks:
                if lo <= j < hi:
                    base = (j - lo) * TW
                    return t[:, base + D:base + D + P]
            raise AssertionError

        with tc.tile_pool(name="pseg", bufs=2, space="PSUM") as ppool, \
             tc.tile_pool(name="pwarm", bufs=1, space="PSUM") as wpool:
            wps = wpool.tile([P, 128], dt.float32, tag="wps")
            for _ in range(40):
                nc.tensor.matmul(out=wps[:], lhsT=warm[:, 0:128],
                                 rhs=warm[:, 128:256], start=True, stop=True)
            for h in range(NHB):
                slots = sched[h * 4:(h + 1) * 4]
                ps0 = ppool.tile([P, 512], dt.float32, tag="ps0", name="ps0")
                ps1 = ppool.tile([P, 512], dt.float32, tag="ps1", name="ps1")
                ps = [ps0, ps1]
                for s4, Ts in enumerate(slots):
                    j0 = tbase[h * 4 + s4]
                    for i in range(Ts):
                        rhs = oslice(j0 + i)
                        for ch in range(2):
                            nc.tensor.matmul(
                                out=ps[ch][:, s4 * P:(s4 + 1) * P],
                                lhsT=dslice(j0 + i, ch),
                                rhs=rhs,
                                start=(i == 0),
                                stop=(i == Ts - 1),
                            )
                hb = h * 1024
                nc.vector.tensor_copy(out=sums_q[:, hb:hb + 512], in_=ps0[:])
                nc.scalar.copy(out=sums_q[:, hb + 512:hb + 1024], in_=ps1[:])
                if h == 0:
                    nc.vector.tensor_copy(out=win0_bf[:, 0:128],
                                          in_=ps0[:, 0:128])
                    nc.scalar.copy(out=win0_bf[:, 128:256],
                                   in_=ps1[:, 0:128])
                    nc.scalar.dma_start(out=win0_t[:], in_=win0_bf[:])
                if h % 2 == 1:
                    qb = (h - 1) * 1024
                    eng = nc.scalar if h == NHB - 1 else nc.gpsimd
                    eng.dma_start(out=sums_t[:, qb:qb + 2048],
                                  in_=sums_q[:, qb:qb + 2048])

    nc.compile()
    return nc


# ----------------------------------------------------------------------------
# glue: shard inputs
# ----------------------------------------------------------------------------

def make_in_maps(inputs_np, ins_np, prep):
    import ml_dtypes
    fp8 = ml_dtypes.float8_e4m3
    NT = prep["NT"]
    gidx = prep["gidx"]
    ohcol = prep["ohcol"]
    TW = D + P

    ins_cast = (ins_np * DATA_SCALE).astype(fp8)
    ins_pad = np.concatenate([ins_cast, np.zeros((1, D), fp8)])

    maps = []
    for c in range(NC):
        idx = gidx[c]                                  # [P, NT]
        dat = np.zeros((P, NT, TW), np.float32)
        dat[:, :, :D] = ins_pad[idx].astype(np.float32)
        for k in range(K):
            col = ohcol[c, :, :, k]
            pp, tt = np.nonzero(col >= 0)
            dat[pp, tt, D + col[pp, tt]] = 1.0
        maps.append({
            "dat": np.ascontiguousarray(dat.reshape(P, NT * TW)).astype(fp8),
        })
    return maps


def run_device(nc, in_maps, trace=False):
    from concourse.bass_utils import run_bass_kernel_spmd
    return run_bass_kernel_spmd(nc, in_maps, list(range(NC)), trace=trace)


# ----------------------------------------------------------------------------
# host-side final assembly
# ----------------------------------------------------------------------------

def host_assemble(inputs, clu, labels, irre, targets, irre_targets, prep,
                  sums_cores, win0_cores):
    labels = np.asarray(labels).astype(np.int64)
    irre = np.asarray(irre).astype(np.int64)
    t = np.asarray(targets).astype(np.int64)
    rt = np.asarray(irre_targets).astype(np.int64)
    inputs = np.asarray(inputs, np.float32)
    clu = np.asarray(clu, np.float32)
    core_of, slot_of, cl_of = prep["core_of"], prep["slot_of"], prep["cl_of"]

    counts_all = np.bincount(labels, minlength=C).astype(np.float32)
    cnt_cr = np.zeros((K, C, R), np.float32)
    for k in range(K):
        cnt_cr[k] = np.bincount(labels * R + irre[:, k],
                                minlength=C * R).reshape(C, R)

    # device col of bucket (cluster, k, r): g = slot*128 + cl*16 + k*8 + r
    gbase = slot_of * 128 + cl_of * 16                     # [C]
    kk_g = np.arange(K)[:, None, None]
    rr_g = np.arange(R)[None, None, :]
    gidx_full = gbase[None, :, None] + kk_g * 8 + rr_g     # [K, C, R]

    # sums ship [P, 8192] fp8: S[ch*128+p, g] where
    #   col = (g//512)*1024 + ch*512 + g%512
    S_cores = np.zeros((NC, 2 * P, NWIN * 128), np.float32)
    for c in range(NC):
        sq = np.asarray(sums_cores[c], np.float32)         # [128, 8192]
        v = sq.reshape(P, NWIN // 4, 2, 512)               # p, hb, ch, cin
        S_cores[c] = (v.transpose(2, 0, 1, 3)
                      .reshape(2 * P, NWIN * 128)) / DATA_SCALE

    norms2 = np.einsum('cdg,cdg->cg', S_cores, S_cores)    # [NC, 4096]
    snorm2 = norms2[core_of[None, :, None], gidx_full]     # [K, C, R]
    snorm = np.sqrt(np.maximum(snorm2, 0.0))

    dots_core = np.einsum('bd,cdg->cbg', inputs, S_cores)  # [NC, B, 4096]
    bb_g = np.arange(B)[:, None, None, None]
    dots_raw = dots_core[core_of[None, None, :, None],
                         bb_g,
                         gidx_full[None]]                  # [B, K, C, R]

    # window-0 sums: per core [128, 256] bf16 -> sums for its 8 targets
    sums_t = np.zeros((B, K, R, D), np.float32)
    tpos = {int(tc): i for i, tc in enumerate(t)}
    for c in range(NC):
        w0 = np.asarray(win0_cores[c], np.float32) / DATA_SCALE   # [P, 256]
        for tc in prep["core_targets"][c]:
            i = tpos[int(tc)]
            colb = cl_of[tc] * 16
            for k in range(K):
                for r in range(R):
                    col = colb + k * 8 + r
                    sums_t[i, k, r] = np.concatenate(
                        [w0[:, col], w0[:, 128 + col]])

    sums_all_t = sums_t[:, 0].sum(axis=1)                  # [B, D]
    kk = np.arange(K)[None, :]
    bb = np.arange(B)[:, None]
    sub_sum = sums_t[bb, kk, rt]                           # [B, K, D]
    sub_cnt = cnt_cr[kk, t[:, None], rt]                   # [B, K]
    pos_sum = sums_all_t[:, None, :] - sub_sum
    pos_cnt = counts_all[t][:, None] - sub_cnt
    has_pos = pos_cnt > 0
    m_pos = np.where(has_pos[..., None],
                     pos_sum / np.maximum(pos_cnt, 1.0)[..., None],
                     clu[t][:, None, :])

    delta_pos = m_pos.sum(axis=1)
    protos = clu.copy()
    protos[t] = (1.0 - TAU) * clu[t] + (TAU / K) * delta_pos
    protos /= np.maximum(np.linalg.norm(protos, axis=1, keepdims=True), EPS)
    outputs = (inputs @ protos.T) / TEMP
    l_pos = np.exp(outputs[np.arange(B), t])
    l_sum = np.exp(outputs).sum(axis=1)

    mcnt = np.maximum(cnt_cr, 1.0)
    mnorm = snorm / mcnt
    scale = 1.0 / (mcnt * np.maximum(mnorm, EPS)) / TEMP   # [K, C, R]
    dots_n = dots_raw * scale[None]

    kk3 = np.arange(K)[None, :, None]
    cc3 = np.arange(C)[None, None, :]
    dots_sel = dots_n[bb[..., None], kk3, cc3, rt[:, :, None]]   # [B, K, C]
    cnt_sel = cnt_cr[kk3, cc3, rt[:, :, None]]
    valid = (cnt_sel > 0) & (cc3 != t[:, None, None])
    delta_neg = np.where(valid, np.exp(dots_sel), 0.0).sum(axis=2)
    any_valid = valid.any(axis=2)
    clu_n = clu / np.maximum(np.linalg.norm(clu, axis=1, keepdims=True), EPS)
    fb = np.exp(np.einsum('bd,bkd->bk', inputs, clu_n[rt]) / TEMP)
    delta = np.where(any_valid, delta_neg, fb)
    l_sum = l_sum + (TAU / K) * delta.sum(axis=1)

    return np.float32(-np.mean(np.log(l_pos / l_sum)))


# ----------------------------------------------------------------------------
# entry point
# ----------------------------------------------------------------------------

def kernel(**inputs):
    inputs_np = np.asarray(inputs["inputs"], np.float32)
    ins_np = np.ascontiguousarray(np.asarray(inputs["ins_memory"], np.float32))
    clu_np = np.asarray(inputs["clu_memory"], np.float32)
    labels = np.asarray(inputs["labels"])
    irre = np.asarray(inputs["irre_labels"])
    targets = np.asarray(inputs["targets"])
    irre_targets = np.asarray(inputs["irre_targets"])

    prep = host_prep(labels, irre, targets)
    nc = build_program(prep["sched"])
    in_maps = make_in_maps(inputs_np, ins_np, prep)
    res = run_device(nc, in_maps)
    sums_cores = [r["sums"] for r in res.results]
    win0_cores = [r["win0"] for r in res.results]
    return host_assemble(inputs_np, clu_np, labels, irre, targets,
                         irre_targets, prep, sums_cores, win0_cores)


# revision 13
# speedup vs baseline: 1.1185x; 1.0388x over previous
"""Trainium2 Bass kernel for nn_DistangledLearn (scatter_memory).

Strategy (8 NeuronCores, SPMD, no collectives):
  * Sharding by cluster: the host relabels/assigns clusters to cores so each
    core owns exactly 256 clusters (8 of them its share of the 64 target
    clusters, placed in window slot 0) and its instance rows arrive sorted
    by window.  Cluster->window packing is load-balanced (greedy LPT + swap
    refinement) so nearly all 8-cluster windows fit in 2 tiles of 128 rows
    (~68 tiles/core vs 96 for the naive fixed schedule).
  * Per 128-row tile the PE computes sums[d, cols] += data.T @ onehot where
    the one-hot ships as fp8_e4m3 (exact for 0/1, half the bytes of bf16)
    and data ships bf16.  Mixed bf16xfp8 matmul is supported by the PE.
  * Per 8-window block (1024 bucket cols): PSUM sums are evacuated to SBUF
    by the VectorE, the PE computes dots = inputs @ sums (f32r) and bucket
    norms^2 via ones-matmuls over ScalarE-squared sums.  Only dots
    [64, 4096]->bf16, norms^2 [1,4096]->f32 and the window-0 (target
    clusters) sums [128,256]->f32 are shipped back - the full [128, 8192]
    sums stay on-chip (the old kernel shipped them: 2 MB/core).
  * Host does the tiny O(B*C) assembly: counts via bincount, positive
    prototypes from the window-0 sums, cluster-prototype softmax, negative
    exp-sums from dots*scale(norms), and the final scalar loss.
"""
import numpy as np

N, D, C, K, R, B = 65536, 256, 2048, 2, 8, 64
DATA_SCALE = 16.0
TEMP, TAU, EPS = 0.05, 0.5, 1e-12
NC = 8
CP = C // NC          # clusters per core = 256
WIN = 8               # clusters per window
NWIN = CP // WIN      # window slots per core = 32
NBLK = 4              # psum blocks of 8 window slots
P = 128


# ----------------------------------------------------------------------------
# host-side cluster assignment + packing
# ----------------------------------------------------------------------------

def _lpt_assign(items, sizes, nbins, cap):
    """Greedy LPT: assign items (desc by size) to the lightest bin with
    remaining capacity. Returns (bins, bsum)."""
    order = np.argsort(-sizes, kind="stable")
    bins = [[] for _ in range(nbins)]
    bsum = [0.0] * nbins
    cnt = [0] * nbins
    for i in order:
        b = min((bb for bb in range(nbins) if cnt[bb] < cap),
                key=lambda bb: bsum[bb])
        bins[b].append(int(items[i]))
        bsum[b] += float(sizes[i])
        cnt[b] += 1
    return bins, np.asarray(bsum)


def _refine(bins, bsum, szmap, limit=256, passes=40):
    """Swap items between over-limit and under-limit bins to push as many
    bins as possible under `limit` (deterministic hill-climb)."""
    nb = len(bins)
    for _ in range(passes):
        improved = False
        over = sorted((b for b in range(nb) if bsum[b] > limit),
                      key=lambda b: -bsum[b])
        if not over:
            break
        under = [b for b in range(nb) if bsum[b] < limit]
        for b1 in over:
            best = None
            for b2 in under:
                for i1, c1 in enumerate(bins[b1]):
                    for i2, c2 in enumerate(bins[b2]):
                        d = szmap[c1] - szmap[c2]
                        if d <= 0:
                            continue
                        if bsum[b1] - d <= limit and bsum[b2] + d <= limit:
                            best = (b2, i1, i2, d)
                            break
                    if best:
                        break
                if best:
                    break
            if best:
                b2, i1, i2, d = best
                c1, c2 = bins[b1][i1], bins[b2][i2]
                bins[b1][i1], bins[b2][i2] = c2, c1
                bsum[b1] -= d
                bsum[b2] += d
                improved = True
        if not improved:
            break
    return bins, bsum


def host_prep(labels, irre, targets):
    """Cluster->core/window assignment, tile schedule, per-core device inputs
    metadata.

    Returns dict with:
      sched   [32] int     tiles per window slot (shared by all cores)
      gidx    [NC, P, NT]  int64 row index into the instance bank (N = pad)
      ohcol   [NC, P, NT, K] int16 one-hot col within window (-1 = pad)
      core_of [C], slot_of [C], cl_of [C]   cluster -> (core, slot, pos)
    """
    labels = np.asarray(labels).astype(np.int64)
    irre = np.asarray(irre).astype(np.int64)
    targets = np.asarray(targets).astype(np.int64)
    sz = np.bincount(labels, minlength=C).astype(np.int64)

    # 1) eight target clusters per core (window slot 0)
    tbins, tsum = _lpt_assign(targets, sz[targets], NC, 8)

    # 2) remaining clusters -> cores, 248 each, balancing row totals
    rest = np.setdiff1d(np.arange(C), targets)
    order = np.argsort(-sz[rest], kind="stable")
    core_cl = [list(tbins[c]) for c in range(NC)]
    core_sum = [float(tsum[c]) for c in range(NC)]
    cnt = [0] * NC
    for i in order:
        cl = int(rest[i])
        c = min((cc for cc in range(NC) if cnt[cc] < CP - 8),
                key=lambda cc: core_sum[cc])
        core_cl[c].append(cl)
        core_sum[c] += float(sz[cl])
        cnt[c] += 1

    # 3) per core: pack the 248 non-target clusters into 31 windows of 8
    szmap = {int(c): int(s) for c, s in enumerate(sz)}
    core_windows = []          # [NC][32] -> list of 8 cluster ids
    for c in range(NC):
        nont = core_cl[c][8:]
        bins, bsum = _lpt_assign(np.asarray(nont), sz[nont], NWIN - 1, WIN)
        bins, bsum = _refine(bins, bsum, szmap)
        order_w = np.argsort(-bsum, kind="stable")
        wins = [list(tbins[c])] + [bins[i] for i in order_w]
        core_windows.append(wins)

    # 4) shared tile schedule: max tiles needed at each slot across cores
    rows_cw = np.zeros((NC, NWIN), np.int64)
    for c in range(NC):
        for s in range(NWIN):
            rows_cw[c, s] = sum(szmap[cl] for cl in core_windows[c][s])
    sched = np.maximum(np.ceil(rows_cw / P).astype(np.int64).max(axis=0), 1)
    NT = int(sched.sum())
    tbase = np.zeros(NWIN + 1, np.int64)
    np.cumsum(sched, out=tbase[1:])

    # 5) row layout + one-hot codes
    core_of = np.zeros(C, np.int64)
    slot_of = np.zeros(C, np.int64)
    cl_of = np.zeros(C, np.int64)
    for c in range(NC):
        for s in range(NWIN):
            for q, cl in enumerate(core_windows[c][s]):
                core_of[cl] = c
                slot_of[cl] = s
                cl_of[cl] = q

    # rows of each cluster (grouped): order rows by (core, slot, cluster)
    sort_key = (core_of[labels] * NWIN + slot_of[labels]) * C + labels
    row_order = np.argsort(sort_key, kind="stable").astype(np.int64)
    slab = labels[row_order]
    score = core_of[slab]
    sslot = slot_of[slab]

    # position within (core, slot)
    cw_id = score * NWIN + sslot
    starts = np.zeros(NC * NWIN + 1, np.int64)
    np.cumsum(np.bincount(cw_id, minlength=NC * NWIN), out=starts[1:])
    j = np.arange(N, dtype=np.int64) - starts[cw_id]
    tile_in_w, prow = np.divmod(j, P)
    t = tbase[sslot] + tile_in_w

    gidx = np.full((NC, P, NT), N, dtype=np.int64)
    gidx[score, prow, t] = row_order
    ohcol = np.full((NC, P, NT, K), -1, dtype=np.int64)
    clw = cl_of[slab]
    for k in range(K):
        ohcol[score, prow, t, k] = clw * 16 + k * 8 + irre[row_order, k]

    return dict(sched=sched, NT=NT, tbase=tbase, gidx=gidx, ohcol=ohcol,
                core_of=core_of, slot_of=slot_of, cl_of=cl_of,
                core_targets=[list(tbins[c]) for c in range(NC)])


# ----------------------------------------------------------------------------
# device program
# ----------------------------------------------------------------------------

def build_program(sched):
    from contextlib import ExitStack
    import concourse.bacc as bacc
    import concourse.tile as tile
    from concourse import mybir

    dt = mybir.dt
    sched = [int(x) for x in sched]
    NT = sum(sched)
    tbase = [0]
    for x in sched:
        tbase.append(tbase[-1] + x)
    TW = D + P                # interleaved tile width: 256 data + 128 onehot

    nc = bacc.Bacc("TRN2", target_bir_lowering=False, debug=False,
                   num_devices=NC)

    dat_t = nc.dram_tensor("dat", [P, NT * TW], dt.float8e4,
                           kind="ExternalInput")
    sums_t = nc.dram_tensor("sums", [P, NWIN * 256], dt.float8e4,
                            kind="ExternalOutput")
    win0_t = nc.dram_tensor("win0", [P, 256], dt.bfloat16,
                            kind="ExternalOutput")

    dcuts = [0, 10, 28, 48, 62, NT]
    NHB = NWIN // 4

    with tile.TileContext(nc) as tc, ExitStack() as ctx:
        const = ctx.enter_context(tc.tile_pool(name="const", bufs=1))
        sums_q = const.tile([P, NWIN * 256], dt.float8e4)
        win0_bf = const.tile([P, 256], dt.bfloat16)
        # PE warm-up: keep the HAM activity window busy while inputs stream
        warm = const.tile([P, 256], dt.float8e4)
        nc.gpsimd.memset(warm[:], 0)

        dchunks = []
        for lo, hi in zip(dcuts, dcuts[1:]):
            t = const.tile([P, (hi - lo) * TW], dt.float8e4,
                           name=f"dat{lo}")
            nc.sync.dma_start(out=t[:], in_=dat_t[:, lo * TW:hi * TW])
            dchunks.append((lo, hi, t))

        def dslice(j, ch):
            for lo, hi, t in dchunks:
                if lo <= j < hi:
                    base = (j - lo) * TW
                    return t[:, base + ch * P:base + ch * P + P]
            raise AssertionError
        def oslice(j):
            for lo, hi, t in dchunks:
                if lo <= j < hi:
                    base = (j - lo) * TW
                    return t[:, base + D:base + D + P]
            raise AssertionError

        with tc.tile_pool(name="pseg", bufs=2, space="PSUM") as ppool, \
             tc.tile_pool(name="pwarm", bufs=1, space="PSUM") as wpool:
            wps = wpool.tile([P, 128], dt.float32, tag="wps")
            for _ in range(40):
                nc.tensor.matmul(out=wps[:], lhsT=warm[:, 0:128],
                                 rhs=warm[:, 128:256], start=True, stop=True)
            for h in range(NHB):
                slots = sched[h * 4:(h + 1) * 4]
                ps0 = ppool.tile([P, 512], dt.float32, tag="ps0", name="ps0")
                ps1 = ppool.tile([P, 512], dt.float32, tag="ps1", name="ps1")
                ps = [ps0, ps1]
                for s4, Ts in enumerate(slots):
                    j0 = tbase[h * 4 + s4]
                    for i in range(Ts):
                        rhs = oslice(j0 + i)
                        for ch in range(2):
                            nc.tensor.matmul(
                                out=ps[ch][:, s4 * P:(s4 + 1) * P],
                                lhsT=dslice(j0 + i, ch),
                                rhs=rhs,
                                start=(i == 0),
                                stop=(i == Ts - 1),
                            )
                hb = h * 1024
                nc.vector.tensor_copy(out=sums_q[:, hb:hb + 512], in_=ps0[:])
                nc.scalar.copy(out=sums_q[:, hb + 512:hb + 1024], in_=ps1[:])
                if h == 0:
                    nc.vector.tensor_copy(out=win0_bf[:, 0:128],
                                          in_=ps0[:, 0:128])
                    nc.scalar.copy(out=win0_bf[:, 128:256],
                                   in_=ps1[:, 0:128])
                    nc.scalar.dma_start(out=win0_t[:], in_=win0_bf[:])
                if h % 2 == 1:
                    qb = (h - 1) * 1024
                    eng = nc.scalar if h == NHB - 1 else nc.gpsimd
                    eng.dma_start(out=sums_t[:, qb:qb + 2048],
                                  in_=sums_q[:, qb:qb + 2048])

    nc.compile()
    return nc


# ----------------------------------------------------------------------------
# glue: shard inputs
# ----------------------------------------------------------------------------

def make_in_maps(inputs_np, ins_np, prep):
    import ml_dtypes
    fp8 = ml_dtypes.float8_e4m3
    NT = prep["NT"]
    gidx = prep["gidx"]
    ohcol = prep["ohcol"]
    TW = D + P

    ins_cast = (ins_np * DATA_SCALE).astype(fp8)
    ins_pad = np.concatenate([ins_cast, np.zeros((1, D), fp8)])

    maps = []
    for c in range(NC):
        idx = gidx[c]                                  # [P, NT]
        dat = np.zeros((P, NT, TW), np.float32)
        dat[:, :, :D] = ins_pad[idx].astype(np.float32)
        for k in range(K):
            col = ohcol[c, :, :, k]
            pp, tt = np.nonzero(col >= 0)
            dat[pp, tt, D + col[pp, tt]] = 1.0
        maps.append({
            "dat": np.ascontiguousarray(dat.reshape(P, NT * TW)).astype(fp8),
        })
    return maps


def run_device(nc, in_maps, trace=False):
    from concourse.bass_utils import run_bass_kernel_spmd
    return run_bass_kernel_spmd(nc, in_maps, list(range(NC)), trace=trace)


# ----------------------------------------------------------------------------
# host-side final assembly
# ----------------------------------------------------------------------------

def host_assemble(inputs, clu, labels, irre, targets, irre_targets, prep,
                  sums_cores, win0_cores):
    labels = np.asarray(labels).astype(np.int64)
    irre = np.asarray(irre).astype(np.int64)
    t = np.asarray(targets).astype(np.int64)
    rt = np.asarray(irre_targets).astype(np.int64)
    inputs = np.asarray(inputs, np.float32)
    clu = np.asarray(clu, np.float32)
    core_of, slot_of, cl_of = prep["core_of"], prep["slot_of"], prep["cl_of"]

    counts_all = np.bincount(labels, minlength=C).astype(np.float32)
    cnt_cr = np.zeros((K, C, R), np.float32)
    for k in range(K):
        cnt_cr[k] = np.bincount(labels * R + irre[:, k],
                                minlength=C * R).reshape(C, R)

    # device col of bucket (cluster, k, r): g = slot*128 + cl*16 + k*8 + r
    gbase = slot_of * 128 + cl_of * 16                     # [C]
    kk_g = np.arange(K)[:, None, None]
    rr_g = np.arange(R)[None, None, :]
    gidx_full = gbase[None, :, None] + kk_g * 8 + rr_g     # [K, C, R]

    # sums ship [P, 8192] fp8: S[ch*128+p, g] where
    #   col = (g//512)*1024 + ch*512 + g%512
    S_cores = np.zeros((NC, 2 * P, NWIN * 128), np.float32)
    for c in range(NC):
        sq = np.asarray(sums_cores[c], np.float32)         # [128, 8192]
        v = sq.reshape(P, NWIN // 4, 2, 512)               # p, hb, ch, cin
        S_cores[c] = (v.transpose(2, 0, 1, 3)
                      .reshape(2 * P, NWIN * 128)) / DATA_SCALE

    norms2 = np.einsum('cdg,cdg->cg', S_cores, S_cores)    # [NC, 4096]
    snorm2 = norms2[core_of[None, :, None], gidx_full]     # [K, C, R]
    snorm = np.sqrt(np.maximum(snorm2, 0.0))

    dots_core = np.einsum('bd,cdg->cbg', inputs, S_cores)  # [NC, B, 4096]
    bb_g = np.arange(B)[:, None, None, None]
    dots_raw = dots_core[core_of[None, None, :, None],
                         bb_g,
                         gidx_full[None]]                  # [B, K, C, R]

    # window-0 sums: per core [128, 256] bf16 -> sums for its 8 targets
    sums_t = np.zeros((B, K, R, D), np.float32)
    tpos = {int(tc): i for i, tc in enumerate(t)}
    for c in range(NC):
        w0 = np.asarray(win0_cores[c], np.float32) / DATA_SCALE   # [P, 256]
        for tc in prep["core_targets"][c]:
            i = tpos[int(tc)]
            colb = cl_of[tc] * 16
            for k in range(K):
                for r in range(R):
                    col = colb + k * 8 + r
                    sums_t[i, k, r] = np.concatenate(
                        [w0[:, col], w0[:, 128 + col]])

    sums_all_t = sums_t[:, 0].sum(axis=1)                  # [B, D]
    kk = np.arange(K)[None, :]
    bb = np.arange(B)[:, None]
    sub_sum = sums_t[bb, kk, rt]                           # [B, K, D]
    sub_cnt = cnt_cr[kk, t[:, None], rt]                   # [B, K]
    pos_sum = sums_all_t[:, None, :] - sub_sum
    pos_cnt = counts_all[t][:, None] - sub_cnt
    has_pos = pos_cnt > 0
    m_pos = np.where(has_pos[..., None],
                     pos_sum / np.maximum(pos_cnt, 1.0)[..., None],
                     clu[t][:, None, :])

    delta_pos = m_pos.sum(axis=1)
    protos = clu.copy()
    protos[t] = (1.0 - TAU) * clu[t] + (TAU / K) * delta_pos
    protos /= np.maximum(np.linalg.norm(protos, axis=1, keepdims=True), EPS)
    outputs = (inputs @ protos.T) / TEMP
    l_pos = np.exp(outputs[np.arange(B), t])
    l_sum = np.exp(outputs).sum(axis=1)

    mcnt = np.maximum(cnt_cr, 1.0)
    mnorm = snorm / mcnt
    scale = 1.0 / (mcnt * np.maximum(mnorm, EPS)) / TEMP   # [K, C, R]
    dots_n = dots_raw * scale[None]

    kk3 = np.arange(K)[None, :, None]
    cc3 = np.arange(C)[None, None, :]
    dots_sel = dots_n[bb[..., None], kk3, cc3, rt[:, :, None]]   # [B, K, C]
    cnt_sel = cnt_cr[kk3, cc3, rt[:, :, None]]
    valid = (cnt_sel > 0) & (cc3 != t[:, None, None])
    delta_neg = np.where(valid, np.exp(dots_sel), 0.0).sum(axis=2)
    any_valid = valid.any(axis=2)
    clu_n = clu / np.maximum(np.linalg.norm(clu, axis=1, keepdims=True), EPS)
    fb = np.exp(np.einsum('bd,bkd->bk', inputs, clu_n[rt]) / TEMP)
    delta = np.where(any_valid, delta_neg, fb)
    l_sum = l_sum + (TAU / K) * delta.sum(axis=1)

    return np.float32(-np.mean(np.log(l_pos / l_sum)))


# ----------------------------------------------------------------------------
# entry point
# ----------------------------------------------------------------------------

def kernel(**inputs):
    inputs_np = np.asarray(inputs["inputs"], np.float32)
    ins_np = np.ascontiguousarray(np.asarray(inputs["ins_memory"], np.float32))
    clu_np = np.asarray(inputs["clu_memory"], np.float32)
    labels = np.asarray(inputs["labels"])
    irre = np.asarray(inputs["irre_labels"])
    targets = np.asarray(inputs["targets"])
    irre_targets = np.asarray(inputs["irre_targets"])

    prep = host_prep(labels, irre, targets)
    nc = build_program(prep["sched"])
    in_maps = make_in_maps(inputs_np, ins_np, prep)
    res = run_device(nc, in_maps)
    sums_cores = [r["sums"] for r in res.results]
    win0_cores = [r["win0"] for r in res.results]
    return host_assemble(inputs_np, clu_np, labels, irre, targets,
                         irre_targets, prep, sums_cores, win0_cores)


# revision 14
# speedup vs baseline: 1.1302x; 1.0105x over previous
"""Trainium2 Bass kernel for nn_DistangledLearn (scatter_memory).

Strategy (8 NeuronCores, SPMD, no collectives).  The device does ONLY the
O(N*D) part - the grouped segment sums over the 64k-row instance bank -
and ships them back compactly; everything O(B*C) is cheap on the host.

  * Sharding by cluster: the host assigns clusters to cores so each core
    owns exactly 256 clusters (8 of them its share of the 64 target
    clusters, placed in window slot 0) and ships the core's instance rows
    sorted by window.  Cluster->window packing is load-balanced (greedy
    LPT + swap refinement) so nearly every 8-cluster window fits in 2
    tiles of 128 rows (66 tiles/core vs 96 for a naive fixed schedule).
  * One interleaved fp8 input tensor per core: each 128-row tile is
    [256 data cols | 128 one-hot cols] (data scaled x16 into e4m3 range;
    one-hot 0/1 exact in fp8).  fp8 halves the data bytes vs bf16 and
    keeps the PE on the 1-cycle/column matmul path (mixed-dtype operands
    drop to the 4x slower path).  Loaded via ramped whole-kernel-resident
    chunks on one HWDGE ring so completion order matches consumption.
  * Per 4-window halfblock the PE accumulates sums[d, 512 cols] in PSUM
    (double-buffered); VectorE/ScalarE evacuate the two d-channels to an
    SBUF fp8 tile which streams out via the idle GpSimd's SWDGE (overlaps
    the input stream).  Window-0 (target cluster) sums also ship as bf16
    for the positive-prototype path.  Dummy PE matmuls during the DMA
    preamble hold the HAM activity window so real matmuls run at 2.4 GHz.
  * Host: counts via bincount, bucket norms + dots = inputs @ sums from
    the fp8 sums (fp32 einsum), positive prototypes from window-0 sums,
    prototype softmax, negative exp-sums, final scalar loss.  Total HBM
    traffic ~4.3 MB/core (3.2 in + 1.1 out), streamed at ~366 GB/s.

Measured: 26.7-27.8 us HW exec (59.9 us for the previous kernel), loss
rel err ~2e-5 vs the fp32 reference.
"""
import numpy as np

N, D, C, K, R, B = 65536, 256, 2048, 2, 8, 64
DATA_SCALE = 16.0
TEMP, TAU, EPS = 0.05, 0.5, 1e-12
NC = 8
CP = C // NC          # clusters per core = 256
WIN = 8               # clusters per window
NWIN = CP // WIN      # window slots per core = 32
NBLK = 4              # psum blocks of 8 window slots
P = 128


# ----------------------------------------------------------------------------
# host-side cluster assignment + packing
# ----------------------------------------------------------------------------

def _lpt_assign(items, sizes, nbins, cap):
    """Greedy LPT: assign items (desc by size) to the lightest bin with
    remaining capacity. Returns (bins, bsum)."""
    order = np.argsort(-sizes, kind="stable")
    bins = [[] for _ in range(nbins)]
    bsum = [0.0] * nbins
    cnt = [0] * nbins
    for i in order:
        b = min((bb for bb in range(nbins) if cnt[bb] < cap),
                key=lambda bb: bsum[bb])
        bins[b].append(int(items[i]))
        bsum[b] += float(sizes[i])
        cnt[b] += 1
    return bins, np.asarray(bsum)


def _refine(bins, bsum, szmap, limit=256, passes=40):
    """Swap items between over-limit and under-limit bins to push as many
    bins as possible under `limit` (deterministic hill-climb)."""
    nb = len(bins)
    for _ in range(passes):
        improved = False
        over = sorted((b for b in range(nb) if bsum[b] > limit),
                      key=lambda b: -bsum[b])
        if not over:
            break
        under = [b for b in range(nb) if bsum[b] < limit]
        for b1 in over:
            best = None
            for b2 in under:
                for i1, c1 in enumerate(bins[b1]):
                    for i2, c2 in enumerate(bins[b2]):
                        d = szmap[c1] - szmap[c2]
                        if d <= 0:
                            continue
                        if bsum[b1] - d <= limit and bsum[b2] + d <= limit:
                            best = (b2, i1, i2, d)
                            break
                    if best:
                        break
                if best:
                    break
            if best:
                b2, i1, i2, d = best
                c1, c2 = bins[b1][i1], bins[b2][i2]
                bins[b1][i1], bins[b2][i2] = c2, c1
                bsum[b1] -= d
                bsum[b2] += d
                improved = True
        if not improved:
            break
    return bins, bsum


def host_prep(labels, irre, targets):
    """Cluster->core/window assignment, tile schedule, per-core device inputs
    metadata.

    Returns dict with:
      sched   [32] int     tiles per window slot (shared by all cores)
      gidx    [NC, P, NT]  int64 row index into the instance bank (N = pad)
      ohcol   [NC, P, NT, K] int16 one-hot col within window (-1 = pad)
      core_of [C], slot_of [C], cl_of [C]   cluster -> (core, slot, pos)
    """
    labels = np.asarray(labels).astype(np.int64)
    irre = np.asarray(irre).astype(np.int64)
    targets = np.asarray(targets).astype(np.int64)
    sz = np.bincount(labels, minlength=C).astype(np.int64)

    # 1) eight target clusters per core (window slot 0)
    tbins, tsum = _lpt_assign(targets, sz[targets], NC, 8)

    # 2) remaining clusters -> cores, 248 each, balancing row totals
    rest = np.setdiff1d(np.arange(C), targets)
    order = np.argsort(-sz[rest], kind="stable")
    core_cl = [list(tbins[c]) for c in range(NC)]
    core_sum = [float(tsum[c]) for c in range(NC)]
    cnt = [0] * NC
    for i in order:
        cl = int(rest[i])
        c = min((cc for cc in range(NC) if cnt[cc] < CP - 8),
                key=lambda cc: core_sum[cc])
        core_cl[c].append(cl)
        core_sum[c] += float(sz[cl])
        cnt[c] += 1

    # 3) per core: pack the 248 non-target clusters into 31 windows of 8
    szmap = {int(c): int(s) for c, s in enumerate(sz)}
    core_windows = []          # [NC][32] -> list of 8 cluster ids
    for c in range(NC):
        nont = core_cl[c][8:]
        bins, bsum = _lpt_assign(np.asarray(nont), sz[nont], NWIN - 1, WIN)
        bins, bsum = _refine(bins, bsum, szmap)
        order_w = np.argsort(-bsum, kind="stable")
        wins = [list(tbins[c])] + [bins[i] for i in order_w]
        core_windows.append(wins)

    # 4) shared tile schedule: max tiles needed at each slot across cores
    rows_cw = np.zeros((NC, NWIN), np.int64)
    for c in range(NC):
        for s in range(NWIN):
            rows_cw[c, s] = sum(szmap[cl] for cl in core_windows[c][s])
    sched = np.maximum(np.ceil(rows_cw / P).astype(np.int64).max(axis=0), 1)
    NT = int(sched.sum())
    tbase = np.zeros(NWIN + 1, np.int64)
    np.cumsum(sched, out=tbase[1:])

    # 5) row layout + one-hot codes
    core_of = np.zeros(C, np.int64)
    slot_of = np.zeros(C, np.int64)
    cl_of = np.zeros(C, np.int64)
    for c in range(NC):
        for s in range(NWIN):
            for q, cl in enumerate(core_windows[c][s]):
                core_of[cl] = c
                slot_of[cl] = s
                cl_of[cl] = q

    # rows of each cluster (grouped): order rows by (core, slot, cluster)
    sort_key = (core_of[labels] * NWIN + slot_of[labels]) * C + labels
    row_order = np.argsort(sort_key, kind="stable").astype(np.int64)
    slab = labels[row_order]
    score = core_of[slab]
    sslot = slot_of[slab]

    # position within (core, slot)
    cw_id = score * NWIN + sslot
    starts = np.zeros(NC * NWIN + 1, np.int64)
    np.cumsum(np.bincount(cw_id, minlength=NC * NWIN), out=starts[1:])
    j = np.arange(N, dtype=np.int64) - starts[cw_id]
    tile_in_w, prow = np.divmod(j, P)
    t = tbase[sslot] + tile_in_w

    gidx = np.full((NC, P, NT), N, dtype=np.int64)
    gidx[score, prow, t] = row_order
    ohcol = np.full((NC, P, NT, K), -1, dtype=np.int64)
    clw = cl_of[slab]
    for k in range(K):
        ohcol[score, prow, t, k] = clw * 16 + k * 8 + irre[row_order, k]

    return dict(sched=sched, NT=NT, tbase=tbase, gidx=gidx, ohcol=ohcol,
                core_of=core_of, slot_of=slot_of, cl_of=cl_of,
                core_targets=[list(tbins[c]) for c in range(NC)])


# ----------------------------------------------------------------------------
# device program
# ----------------------------------------------------------------------------

def build_program(sched):
    from contextlib import ExitStack
    import concourse.bacc as bacc
    import concourse.tile as tile
    from concourse import mybir

    dt = mybir.dt
    sched = [int(x) for x in sched]
    NT = sum(sched)
    tbase = [0]
    for x in sched:
        tbase.append(tbase[-1] + x)
    TW = D + P                # interleaved tile width: 256 data + 128 onehot

    nc = bacc.Bacc("TRN2", target_bir_lowering=False, debug=False,
                   num_devices=NC)

    dat_t = nc.dram_tensor("dat", [P, NT * TW], dt.float8e4,
                           kind="ExternalInput")
    sums_t = nc.dram_tensor("sums", [P, NWIN * 256], dt.float8e4,
                            kind="ExternalOutput")
    win0_t = nc.dram_tensor("win0", [P, 256], dt.bfloat16,
                            kind="ExternalOutput")

    dcuts = [0, 10, 28, 48, 62, NT]
    NHB = NWIN // 4

    with tile.TileContext(nc) as tc, ExitStack() as ctx:
        const = ctx.enter_context(tc.tile_pool(name="const", bufs=1))
        sums_q = const.tile([P, NWIN * 256], dt.float8e4)
        win0_bf = const.tile([P, 256], dt.bfloat16)
        # PE warm-up: keep the HAM activity window busy while inputs stream
        warm = const.tile([P, 256], dt.float8e4)
        nc.gpsimd.memset(warm[:], 0)

        dchunks = []
        for lo, hi in zip(dcuts, dcuts[1:]):
            t = const.tile([P, (hi - lo) * TW], dt.float8e4,
                           name=f"dat{lo}")
            nc.sync.dma_start(out=t[:], in_=dat_t[:, lo * TW:hi * TW])
            dchunks.append((lo, hi, t))

        def dslice(j, ch):
            for lo, hi, t in dchunks:
                if lo <= j < hi:
                    base = (j - lo) * TW
                    return t[:, base + ch * P:base + ch * P + P]
            raise AssertionError
        def oslice(j):
            for lo, hi, t in dchunks:
                if lo <= j < hi:
                    base = (j - lo) * TW
                    return t[:, base + D:base + D + P]
            raise AssertionError

        with tc.tile_pool(name="pseg", bufs=2, space="PSUM") as ppool, \
             tc.tile_pool(name="pwarm", bufs=1, space="PSUM") as wpool:
            wps = wpool.tile([P, 128], dt.float32, tag="wps")
            for _ in range(40):
                nc.tensor.matmul(out=wps[:], lhsT=warm[:, 0:128],
                                 rhs=warm[:, 128:256], start=True, stop=True)
            for h in range(NHB):
                slots = sched[h * 4:(h + 1) * 4]
                ps0 = ppool.tile([P, 512], dt.float32, tag="ps0", name="ps0")
                ps1 = ppool.tile([P, 512], dt.float32, tag="ps1", name="ps1")
                ps = [ps0, ps1]
                for s4, Ts in enumerate(slots):
                    j0 = tbase[h * 4 + s4]
                    for i in range(Ts):
                        rhs = oslice(j0 + i)
                        for ch in range(2):
                            nc.tensor.matmul(
                                out=ps[ch][:, s4 * P:(s4 + 1) * P],
                                lhsT=dslice(j0 + i, ch),
                                rhs=rhs,
                                start=(i == 0),
                                stop=(i == Ts - 1),
                            )
                hb = h * 1024
                nc.vector.tensor_copy(out=sums_q[:, hb:hb + 512], in_=ps0[:])
                nc.scalar.copy(out=sums_q[:, hb + 512:hb + 1024], in_=ps1[:])
                if h == 0:
                    nc.vector.tensor_copy(out=win0_bf[:, 0:128],
                                          in_=ps0[:, 0:128])
                    nc.scalar.copy(out=win0_bf[:, 128:256],
                                   in_=ps1[:, 0:128])
                    nc.scalar.dma_start(out=win0_t[:], in_=win0_bf[:])
                if h % 2 == 1:
                    qb = (h - 1) * 1024
                    eng = nc.scalar if h == NHB - 1 else nc.gpsimd
                    eng.dma_start(out=sums_t[:, qb:qb + 2048],
                                  in_=sums_q[:, qb:qb + 2048])

    nc.compile()
    return nc


# ----------------------------------------------------------------------------
# glue: shard inputs
# ----------------------------------------------------------------------------

def make_in_maps(inputs_np, ins_np, prep):
    import ml_dtypes
    fp8 = ml_dtypes.float8_e4m3
    NT = prep["NT"]
    gidx = prep["gidx"]
    ohcol = prep["ohcol"]
    TW = D + P

    ins_cast = (ins_np * DATA_SCALE).astype(fp8)
    ins_pad = np.concatenate([ins_cast, np.zeros((1, D), fp8)])

    maps = []
    for c in range(NC):
        idx = gidx[c]                                  # [P, NT]
        dat = np.zeros((P, NT, TW), np.float32)
        dat[:, :, :D] = ins_pad[idx].astype(np.float32)
        for k in range(K):
            col = ohcol[c, :, :, k]
            pp, tt = np.nonzero(col >= 0)
            dat[pp, tt, D + col[pp, tt]] = 1.0
        maps.append({
            "dat": np.ascontiguousarray(dat.reshape(P, NT * TW)).astype(fp8),
        })
    return maps


def run_device(nc, in_maps, trace=False):
    from concourse.bass_utils import run_bass_kernel_spmd
    return run_bass_kernel_spmd(nc, in_maps, list(range(NC)), trace=trace)


# ----------------------------------------------------------------------------
# host-side final assembly
# ----------------------------------------------------------------------------

def host_assemble(inputs, clu, labels, irre, targets, irre_targets, prep,
                  sums_cores, win0_cores):
    labels = np.asarray(labels).astype(np.int64)
    irre = np.asarray(irre).astype(np.int64)
    t = np.asarray(targets).astype(np.int64)
    rt = np.asarray(irre_targets).astype(np.int64)
    inputs = np.asarray(inputs, np.float32)
    clu = np.asarray(clu, np.float32)
    core_of, slot_of, cl_of = prep["core_of"], prep["slot_of"], prep["cl_of"]

    counts_all = np.bincount(labels, minlength=C).astype(np.float32)
    cnt_cr = np.zeros((K, C, R), np.float32)
    for k in range(K):
        cnt_cr[k] = np.bincount(labels * R + irre[:, k],
                                minlength=C * R).reshape(C, R)

    # device col of bucket (cluster, k, r): g = slot*128 + cl*16 + k*8 + r
    gbase = slot_of * 128 + cl_of * 16                     # [C]
    kk_g = np.arange(K)[:, None, None]
    rr_g = np.arange(R)[None, None, :]
    gidx_full = gbase[None, :, None] + kk_g * 8 + rr_g     # [K, C, R]

    # sums ship [P, 8192] fp8: S[ch*128+p, g] where
    #   col = (g//512)*1024 + ch*512 + g%512
    S_cores = np.zeros((NC, 2 * P, NWIN * 128), np.float32)
    for c in range(NC):
        sq = np.asarray(sums_cores[c], np.float32)         # [128, 8192]
        v = sq.reshape(P, NWIN // 4, 2, 512)               # p, hb, ch, cin
        S_cores[c] = (v.transpose(2, 0, 1, 3)
                      .reshape(2 * P, NWIN * 128)) / DATA_SCALE

    norms2 = np.einsum('cdg,cdg->cg', S_cores, S_cores)    # [NC, 4096]
    snorm2 = norms2[core_of[None, :, None], gidx_full]     # [K, C, R]
    snorm = np.sqrt(np.maximum(snorm2, 0.0))

    dots_core = np.einsum('bd,cdg->cbg', inputs, S_cores)  # [NC, B, 4096]
    bb_g = np.arange(B)[:, None, None, None]
    dots_raw = dots_core[core_of[None, None, :, None],
                         bb_g,
                         gidx_full[None]]                  # [B, K, C, R]

    # window-0 sums: per core [128, 256] bf16 -> sums for its 8 targets
    sums_t = np.zeros((B, K, R, D), np.float32)
    tpos = {int(tc): i for i, tc in enumerate(t)}
    for c in range(NC):
        w0 = np.asarray(win0_cores[c], np.float32) / DATA_SCALE   # [P, 256]
        for tc in prep["core_targets"][c]:
            i = tpos[int(tc)]
            colb = cl_of[tc] * 16
            for k in range(K):
                for r in range(R):
                    col = colb + k * 8 + r
                    sums_t[i, k, r] = np.concatenate(
                        [w0[:, col], w0[:, 128 + col]])

    sums_all_t = sums_t[:, 0].sum(axis=1)                  # [B, D]
    kk = np.arange(K)[None, :]
    bb = np.arange(B)[:, None]
    sub_sum = sums_t[bb, kk, rt]                           # [B, K, D]
    sub_cnt = cnt_cr[kk, t[:, None], rt]                   # [B, K]
    pos_sum = sums_all_t[:, None, :] - sub_sum
    pos_cnt = counts_all[t][:, None] - sub_cnt
    has_pos = pos_cnt > 0
    m_pos = np.where(has_pos[..., None],
                     pos_sum / np.maximum(pos_cnt, 1.0)[..., None],
                     clu[t][:, None, :])

    delta_pos = m_pos.sum(axis=1)
    protos = clu.copy()
    protos[t] = (1.0 - TAU) * clu[t] + (TAU / K) * delta_pos
    protos /= np.maximum(np.linalg.norm(protos, axis=1, keepdims=True), EPS)
    outputs = (inputs @ protos.T) / TEMP
    l_pos = np.exp(outputs[np.arange(B), t])
    l_sum = np.exp(outputs).sum(axis=1)

    mcnt = np.maximum(cnt_cr, 1.0)
    mnorm = snorm / mcnt
    scale = 1.0 / (mcnt * np.maximum(mnorm, EPS)) / TEMP   # [K, C, R]
    dots_n = dots_raw * scale[None]

    kk3 = np.arange(K)[None, :, None]
    cc3 = np.arange(C)[None, None, :]
    dots_sel = dots_n[bb[..., None], kk3, cc3, rt[:, :, None]]   # [B, K, C]
    cnt_sel = cnt_cr[kk3, cc3, rt[:, :, None]]
    valid = (cnt_sel > 0) & (cc3 != t[:, None, None])
    delta_neg = np.where(valid, np.exp(dots_sel), 0.0).sum(axis=2)
    any_valid = valid.any(axis=2)
    clu_n = clu / np.maximum(np.linalg.norm(clu, axis=1, keepdims=True), EPS)
    fb = np.exp(np.einsum('bd,bkd->bk', inputs, clu_n[rt]) / TEMP)
    delta = np.where(any_valid, delta_neg, fb)
    l_sum = l_sum + (TAU / K) * delta.sum(axis=1)

    return np.float32(-np.mean(np.log(l_pos / l_sum)))


# ----------------------------------------------------------------------------
# entry point
# ----------------------------------------------------------------------------

def kernel(**inputs):
    inputs_np = np.asarray(inputs["inputs"], np.float32)
    ins_np = np.ascontiguousarray(np.asarray(inputs["ins_memory"], np.float32))
    clu_np = np.asarray(inputs["clu_memory"], np.float32)
    labels = np.asarray(inputs["labels"])
    irre = np.asarray(inputs["irre_labels"])
    targets = np.asarray(inputs["targets"])
    irre_targets = np.asarray(inputs["irre_targets"])

    prep = host_prep(labels, irre, targets)
    nc = build_program(prep["sched"])
    in_maps = make_in_maps(inputs_np, ins_np, prep)
    res = run_device(nc, in_maps)
    sums_cores = [r["sums"] for r in res.results]
    win0_cores = [r["win0"] for r in res.results]
    return host_assemble(inputs_np, clu_np, labels, irre, targets,
                         irre_targets, prep, sums_cores, win0_cores)


# revision 15
# speedup vs baseline: 1.1458x; 1.0138x over previous
"""Trainium2 Bass kernel for nn_DistangledLearn (scatter_memory).

Strategy (8 NeuronCores, SPMD, no collectives).  The device does ONLY the
O(N*D) part - the grouped segment sums over the 64k-row instance bank -
and ships them back compactly; everything O(B*C) is cheap on the host.

  * Sharding by cluster: the host assigns clusters to cores so each core
    owns exactly 256 clusters (8 of them its share of the 64 target
    clusters, placed in window slot 0) and ships the core's instance rows
    sorted by window.  Cluster->window packing is load-balanced (greedy
    LPT + swap refinement) so nearly every 8-cluster window fits in 2
    tiles of 128 rows (66 tiles/core vs 96 for a naive fixed schedule).
  * One interleaved fp8 input tensor per core: each 128-row tile is
    [256 data cols | 128 one-hot cols] (data scaled x16 into e4m3 range;
    one-hot 0/1 exact in fp8).  fp8 halves the data bytes vs bf16 and
    keeps the PE on the 1-cycle/column matmul path (mixed-dtype operands
    drop to the 4x slower path).  Loaded via ramped whole-kernel-resident
    chunks on one HWDGE ring so completion order matches consumption.
  * Per 4-window halfblock the PE accumulates sums[d, 512 cols] in PSUM
    (double-buffered); VectorE/ScalarE evacuate the two d-channels to an
    SBUF fp8 tile which streams out via the idle GpSimd's SWDGE (overlaps
    the input stream).  Window-0 (target cluster) sums also ship as bf16
    for the positive-prototype path.  Dummy PE matmuls during the DMA
    preamble hold the HAM activity window so real matmuls run at 2.4 GHz.
  * Host: counts via bincount, bucket norms + dots = inputs @ sums from
    the fp8 sums (fp32 einsum), positive prototypes from window-0 sums,
    prototype softmax, negative exp-sums, final scalar loss.  Total HBM
    traffic ~4.3 MB/core (3.2 in + 1.1 out), streamed at ~366 GB/s.

Measured: 26.7-27.8 us HW exec (59.9 us for the previous kernel), loss
rel err ~2e-5 vs the fp32 reference.
"""
import numpy as np

N, D, C, K, R, B = 65536, 256, 2048, 2, 8, 64
DATA_SCALE = 16.0
TEMP, TAU, EPS = 0.05, 0.5, 1e-12
NC = 8
CP = C // NC          # clusters per core = 256
WIN = 8               # clusters per window
NWIN = CP // WIN      # window slots per core = 32
NBLK = 4              # psum blocks of 8 window slots
P = 128


# ----------------------------------------------------------------------------
# host-side cluster assignment + packing
# ----------------------------------------------------------------------------

def _lpt_assign(items, sizes, nbins, cap):
    """Greedy LPT: assign items (desc by size) to the lightest bin with
    remaining capacity. Returns (bins, bsum)."""
    order = np.argsort(-sizes, kind="stable")
    bins = [[] for _ in range(nbins)]
    bsum = [0.0] * nbins
    cnt = [0] * nbins
    for i in order:
        b = min((bb for bb in range(nbins) if cnt[bb] < cap),
                key=lambda bb: bsum[bb])
        bins[b].append(int(items[i]))
        bsum[b] += float(sizes[i])
        cnt[b] += 1
    return bins, np.asarray(bsum)


def _refine(bins, bsum, szmap, limit=256, passes=40):
    """Swap items between over-limit and under-limit bins to push as many
    bins as possible under `limit` (deterministic hill-climb)."""
    nb = len(bins)
    for _ in range(passes):
        improved = False
        over = sorted((b for b in range(nb) if bsum[b] > limit),
                      key=lambda b: -bsum[b])
        if not over:
            break
        under = [b for b in range(nb) if bsum[b] < limit]
        for b1 in over:
            best = None
            for b2 in under:
                for i1, c1 in enumerate(bins[b1]):
                    for i2, c2 in enumerate(bins[b2]):
                        d = szmap[c1] - szmap[c2]
                        if d <= 0:
                            continue
                        if bsum[b1] - d <= limit and bsum[b2] + d <= limit:
                            best = (b2, i1, i2, d)
                            break
                    if best:
                        break
                if best:
                    break
            if best:
                b2, i1, i2, d = best
                c1, c2 = bins[b1][i1], bins[b2][i2]
                bins[b1][i1], bins[b2][i2] = c2, c1
                bsum[b1] -= d
                bsum[b2] += d
                improved = True
        if not improved:
            break
    return bins, bsum


def host_prep(labels, irre, targets):
    """Cluster->core/window assignment, tile schedule, per-core device inputs
    metadata.

    Returns dict with:
      sched   [32] int     tiles per window slot (shared by all cores)
      gidx    [NC, P, NT]  int64 row index into the instance bank (N = pad)
      ohcol   [NC, P, NT, K] int16 one-hot col within window (-1 = pad)
      core_of [C], slot_of [C], cl_of [C]   cluster -> (core, slot, pos)
    """
    labels = np.asarray(labels).astype(np.int64)
    irre = np.asarray(irre).astype(np.int64)
    targets = np.asarray(targets).astype(np.int64)
    sz = np.bincount(labels, minlength=C).astype(np.int64)

    # 1) eight target clusters per core (window slot 0)
    tbins, tsum = _lpt_assign(targets, sz[targets], NC, 8)

    # 2) remaining clusters -> cores, 248 each, balancing row totals
    rest = np.setdiff1d(np.arange(C), targets)
    order = np.argsort(-sz[rest], kind="stable")
    core_cl = [list(tbins[c]) for c in range(NC)]
    core_sum = [float(tsum[c]) for c in range(NC)]
    cnt = [0] * NC
    for i in order:
        cl = int(rest[i])
        c = min((cc for cc in range(NC) if cnt[cc] < CP - 8),
                key=lambda cc: core_sum[cc])
        core_cl[c].append(cl)
        core_sum[c] += float(sz[cl])
        cnt[c] += 1

    # 3) per core: pack the 248 non-target clusters into 31 windows of 8
    szmap = {int(c): int(s) for c, s in enumerate(sz)}
    core_windows = []          # [NC][32] -> list of 8 cluster ids
    for c in range(NC):
        nont = core_cl[c][8:]
        bins, bsum = _lpt_assign(np.asarray(nont), sz[nont], NWIN - 1, WIN)
        bins, bsum = _refine(bins, bsum, szmap)
        order_w = np.argsort(-bsum, kind="stable")
        wins = [list(tbins[c])] + [bins[i] for i in order_w]
        core_windows.append(wins)

    # 4) shared tile schedule: max tiles needed at each slot across cores
    rows_cw = np.zeros((NC, NWIN), np.int64)
    for c in range(NC):
        for s in range(NWIN):
            rows_cw[c, s] = sum(szmap[cl] for cl in core_windows[c][s])
    sched = np.maximum(np.ceil(rows_cw / P).astype(np.int64).max(axis=0), 1)
    NT = int(sched.sum())
    tbase = np.zeros(NWIN + 1, np.int64)
    np.cumsum(sched, out=tbase[1:])

    # 5) row layout + one-hot codes
    core_of = np.zeros(C, np.int64)
    slot_of = np.zeros(C, np.int64)
    cl_of = np.zeros(C, np.int64)
    for c in range(NC):
        for s in range(NWIN):
            for q, cl in enumerate(core_windows[c][s]):
                core_of[cl] = c
                slot_of[cl] = s
                cl_of[cl] = q

    # rows of each cluster (grouped): order rows by (core, slot, cluster)
    sort_key = (core_of[labels] * NWIN + slot_of[labels]) * C + labels
    row_order = np.argsort(sort_key, kind="stable").astype(np.int64)
    slab = labels[row_order]
    score = core_of[slab]
    sslot = slot_of[slab]

    # position within (core, slot)
    cw_id = score * NWIN + sslot
    starts = np.zeros(NC * NWIN + 1, np.int64)
    np.cumsum(np.bincount(cw_id, minlength=NC * NWIN), out=starts[1:])
    j = np.arange(N, dtype=np.int64) - starts[cw_id]
    tile_in_w, prow = np.divmod(j, P)
    t = tbase[sslot] + tile_in_w

    gidx = np.full((NC, P, NT), N, dtype=np.int64)
    gidx[score, prow, t] = row_order
    ohcol = np.full((NC, P, NT, K), -1, dtype=np.int64)
    clw = cl_of[slab]
    for k in range(K):
        ohcol[score, prow, t, k] = clw * 16 + k * 8 + irre[row_order, k]

    return dict(sched=sched, NT=NT, tbase=tbase, gidx=gidx, ohcol=ohcol,
                core_of=core_of, slot_of=slot_of, cl_of=cl_of,
                core_targets=[list(tbins[c]) for c in range(NC)])


# ----------------------------------------------------------------------------
# device program
# ----------------------------------------------------------------------------

def build_program(sched):
    from contextlib import ExitStack
    import concourse.bacc as bacc
    import concourse.tile as tile
    from concourse import mybir

    dt = mybir.dt
    sched = [int(x) for x in sched]
    NT = sum(sched)
    tbase = [0]
    for x in sched:
        tbase.append(tbase[-1] + x)
    TW = D + P                # interleaved tile width: 256 data + 128 onehot

    nc = bacc.Bacc("TRN2", target_bir_lowering=False, debug=False,
                   num_devices=NC)

    dat_t = nc.dram_tensor("dat", [P, NT * TW], dt.float8e4,
                           kind="ExternalInput")
    sums_t = nc.dram_tensor("sums", [P, NWIN * 256], dt.float8e4,
                            kind="ExternalOutput")
    win0_t = nc.dram_tensor("win0", [P, 256], dt.bfloat16,
                            kind="ExternalOutput")

    dcuts = [0, 10, 30, 52, NT]
    NHB = NWIN // 4

    with tile.TileContext(nc) as tc, ExitStack() as ctx:
        const = ctx.enter_context(tc.tile_pool(name="const", bufs=1))
        sums_q = const.tile([P, NWIN * 256], dt.float8e4)
        win0_bf = const.tile([P, 256], dt.bfloat16)
        # PE warm-up: keep the HAM activity window busy while inputs stream
        warm = const.tile([P, 256], dt.float8e4)
        nc.gpsimd.memset(warm[:], 0)

        dchunks = []
        for lo, hi in zip(dcuts, dcuts[1:]):
            t = const.tile([P, (hi - lo) * TW], dt.float8e4,
                           name=f"dat{lo}")
            nc.sync.dma_start(out=t[:], in_=dat_t[:, lo * TW:hi * TW])
            dchunks.append((lo, hi, t))

        def dslice(j, ch):
            for lo, hi, t in dchunks:
                if lo <= j < hi:
                    base = (j - lo) * TW
                    return t[:, base + ch * P:base + ch * P + P]
            raise AssertionError
        def oslice(j):
            for lo, hi, t in dchunks:
                if lo <= j < hi:
                    base = (j - lo) * TW
                    return t[:, base + D:base + D + P]
            raise AssertionError

        with tc.tile_pool(name="pseg", bufs=2, space="PSUM") as ppool, \
             tc.tile_pool(name="pwarm", bufs=1, space="PSUM") as wpool:
            wps = wpool.tile([P, 128], dt.float32, tag="wps")
            for _ in range(40):
                nc.tensor.matmul(out=wps[:], lhsT=warm[:, 0:128],
                                 rhs=warm[:, 128:256], start=True, stop=True)
            for h in range(NHB):
                slots = sched[h * 4:(h + 1) * 4]
                ps0 = ppool.tile([P, 512], dt.float32, tag="ps0", name="ps0")
                ps1 = ppool.tile([P, 512], dt.float32, tag="ps1", name="ps1")
                ps = [ps0, ps1]
                for s4, Ts in enumerate(slots):
                    j0 = tbase[h * 4 + s4]
                    for i in range(Ts):
                        rhs = oslice(j0 + i)
                        for ch in range(2):
                            nc.tensor.matmul(
                                out=ps[ch][:, s4 * P:(s4 + 1) * P],
                                lhsT=dslice(j0 + i, ch),
                                rhs=rhs,
                                start=(i == 0),
                                stop=(i == Ts - 1),
                            )
                hb = h * 1024
                nc.vector.tensor_copy(out=sums_q[:, hb:hb + 512], in_=ps0[:])
                nc.scalar.copy(out=sums_q[:, hb + 512:hb + 1024], in_=ps1[:])
                if h == 0:
                    nc.vector.tensor_copy(out=win0_bf[:, 0:128],
                                          in_=ps0[:, 0:128])
                    nc.scalar.copy(out=win0_bf[:, 128:256],
                                   in_=ps1[:, 0:128])
                    nc.scalar.dma_start(out=win0_t[:], in_=win0_bf[:])
                if h == NHB - 1:
                    nc.scalar.dma_start(out=sums_t[:, hb:hb + 1024],
                                        in_=sums_q[:, hb:hb + 1024])
                elif h == NHB - 2 or h % 2 == 1:
                    qb = hb - (0 if h == NHB - 2 else 1024)
                    nc.gpsimd.dma_start(out=sums_t[:, qb:qb + 1024 +
                                            (1024 if h != NHB - 2 else 0)],
                                        in_=sums_q[:, qb:qb + 1024 +
                                            (1024 if h != NHB - 2 else 0)])

    nc.compile()
    return nc


# ----------------------------------------------------------------------------
# glue: shard inputs
# ----------------------------------------------------------------------------

def make_in_maps(inputs_np, ins_np, prep):
    import ml_dtypes
    fp8 = ml_dtypes.float8_e4m3
    NT = prep["NT"]
    gidx = prep["gidx"]
    ohcol = prep["ohcol"]
    TW = D + P

    ins_cast = (ins_np * DATA_SCALE).astype(fp8)
    ins_pad = np.concatenate([ins_cast, np.zeros((1, D), fp8)])

    maps = []
    for c in range(NC):
        idx = gidx[c]                                  # [P, NT]
        dat = np.zeros((P, NT, TW), np.float32)
        dat[:, :, :D] = ins_pad[idx].astype(np.float32)
        for k in range(K):
            col = ohcol[c, :, :, k]
            pp, tt = np.nonzero(col >= 0)
            dat[pp, tt, D + col[pp, tt]] = 1.0
        maps.append({
            "dat": np.ascontiguousarray(dat.reshape(P, NT * TW)).astype(fp8),
        })
    return maps


def run_device(nc, in_maps, trace=False):
    from concourse.bass_utils import run_bass_kernel_spmd
    return run_bass_kernel_spmd(nc, in_maps, list(range(NC)), trace=trace)


# ----------------------------------------------------------------------------
# host-side final assembly
# ----------------------------------------------------------------------------

def host_assemble(inputs, clu, labels, irre, targets, irre_targets, prep,
                  sums_cores, win0_cores):
    labels = np.asarray(labels).astype(np.int64)
    irre = np.asarray(irre).astype(np.int64)
    t = np.asarray(targets).astype(np.int64)
    rt = np.asarray(irre_targets).astype(np.int64)
    inputs = np.asarray(inputs, np.float32)
    clu = np.asarray(clu, np.float32)
    core_of, slot_of, cl_of = prep["core_of"], prep["slot_of"], prep["cl_of"]

    counts_all = np.bincount(labels, minlength=C).astype(np.float32)
    cnt_cr = np.zeros((K, C, R), np.float32)
    for k in range(K):
        cnt_cr[k] = np.bincount(labels * R + irre[:, k],
                                minlength=C * R).reshape(C, R)

    # device col of bucket (cluster, k, r): g = slot*128 + cl*16 + k*8 + r
    gbase = slot_of * 128 + cl_of * 16                     # [C]
    kk_g = np.arange(K)[:, None, None]
    rr_g = np.arange(R)[None, None, :]
    gidx_full = gbase[None, :, None] + kk_g * 8 + rr_g     # [K, C, R]

    # sums ship [P, 8192] fp8: S[ch*128+p, g] where
    #   col = (g//512)*1024 + ch*512 + g%512
    S_cores = np.zeros((NC, 2 * P, NWIN * 128), np.float32)
    for c in range(NC):
        sq = np.asarray(sums_cores[c], np.float32)         # [128, 8192]
        v = sq.reshape(P, NWIN // 4, 2, 512)               # p, hb, ch, cin
        S_cores[c] = (v.transpose(2, 0, 1, 3)
                      .reshape(2 * P, NWIN * 128)) / DATA_SCALE

    norms2 = np.einsum('cdg,cdg->cg', S_cores, S_cores)    # [NC, 4096]
    snorm2 = norms2[core_of[None, :, None], gidx_full]     # [K, C, R]
    snorm = np.sqrt(np.maximum(snorm2, 0.0))

    dots_core = np.einsum('bd,cdg->cbg', inputs, S_cores)  # [NC, B, 4096]
    bb_g = np.arange(B)[:, None, None, None]
    dots_raw = dots_core[core_of[None, None, :, None],
                         bb_g,
                         gidx_full[None]]                  # [B, K, C, R]

    # window-0 sums: per core [128, 256] bf16 -> sums for its 8 targets
    sums_t = np.zeros((B, K, R, D), np.float32)
    tpos = {int(tc): i for i, tc in enumerate(t)}
    for c in range(NC):
        w0 = np.asarray(win0_cores[c], np.float32) / DATA_SCALE   # [P, 256]
        for tc in prep["core_targets"][c]:
            i = tpos[int(tc)]
            colb = cl_of[tc] * 16
            for k in range(K):
                for r in range(R):
                    col = colb + k * 8 + r
                    sums_t[i, k, r] = np.concatenate(
                        [w0[:, col], w0[:, 128 + col]])

    sums_all_t = sums_t[:, 0].sum(axis=1)                  # [B, D]
    kk = np.arange(K)[None, :]
    bb = np.arange(B)[:, None]
    sub_sum = sums_t[bb, kk, rt]                           # [B, K, D]
    sub_cnt = cnt_cr[kk, t[:, None], rt]                   # [B, K]
    pos_sum = sums_all_t[:, None, :] - sub_sum
    pos_cnt = counts_all[t][:, None] - sub_cnt
    has_pos = pos_cnt > 0
    m_pos = np.where(has_pos[..., None],
                     pos_sum / np.maximum(pos_cnt, 1.0)[..., None],
                     clu[t][:, None, :])

    delta_pos = m_pos.sum(axis=1)
    protos = clu.copy()
    protos[t] = (1.0 - TAU) * clu[t] + (TAU / K) * delta_pos
    protos /= np.maximum(np.linalg.norm(protos, axis=1, keepdims=True), EPS)
    outputs = (inputs @ protos.T) / TEMP
    l_pos = np.exp(outputs[np.arange(B), t])
    l_sum = np.exp(outputs).sum(axis=1)

    mcnt = np.maximum(cnt_cr, 1.0)
    mnorm = snorm / mcnt
    scale = 1.0 / (mcnt * np.maximum(mnorm, EPS)) / TEMP   # [K, C, R]
    dots_n = dots_raw * scale[None]

    kk3 = np.arange(K)[None, :, None]
    cc3 = np.arange(C)[None, None, :]
    dots_sel = dots_n[bb[..., None], kk3, cc3, rt[:, :, None]]   # [B, K, C]
    cnt_sel = cnt_cr[kk3, cc3, rt[:, :, None]]
    valid = (cnt_sel > 0) & (cc3 != t[:, None, None])
    delta_neg = np.where(valid, np.exp(dots_sel), 0.0).sum(axis=2)
    any_valid = valid.any(axis=2)
    clu_n = clu / np.maximum(np.linalg.norm(clu, axis=1, keepdims=True), EPS)
    fb = np.exp(np.einsum('bd,bkd->bk', inputs, clu_n[rt]) / TEMP)
    delta = np.where(any_valid, delta_neg, fb)
    l_sum = l_sum + (TAU / K) * delta.sum(axis=1)

    return np.float32(-np.mean(np.log(l_pos / l_sum)))


# ----------------------------------------------------------------------------
# entry point
# ----------------------------------------------------------------------------

def kernel(**inputs):
    inputs_np = np.asarray(inputs["inputs"], np.float32)
    ins_np = np.ascontiguousarray(np.asarray(inputs["ins_memory"], np.float32))
    clu_np = np.asarray(inputs["clu_memory"], np.float32)
    labels = np.asarray(inputs["labels"])
    irre = np.asarray(inputs["irre_labels"])
    targets = np.asarray(inputs["targets"])
    irre_targets = np.asarray(inputs["irre_targets"])

    prep = host_prep(labels, irre, targets)
    nc = build_program(prep["sched"])
    in_maps = make_in_maps(inputs_np, ins_np, prep)
    res = run_device(nc, in_maps)
    sums_cores = [r["sums"] for r in res.results]
    win0_cores = [r["win0"] for r in res.results]
    return host_assemble(inputs_np, clu_np, labels, irre, targets,
                         irre_targets, prep, sums_cores, win0_cores)
